# revision 20
# baseline (speedup 1.0000x reference)
"""Trainium2 Bass kernel for nn_Encoding_layer (highway stack + pairwise MLP
attention + fuse gates).

Sharding: data-parallel over batch B=16 across 8 NeuronCores (2 batches per
core); all dense weights replicated. No collectives.

v3: fp8-e4m3 DoubleRow matmuls for the compute-heavy GEMMs, with the
schedule restructured to keep the PE HAM clock-gate warm:
  - DoubleRow contracts 256 rows/pass (2 fp8 weights per PE cell); operand
    pairs are adjacent k-tiles in the free dim of the [128, KU, N] tilings.
  - Quantization (numpy-validated, rel err ~3e-3 vs 2e-2 budget):
      highway (x fp8, W fp8*32)   scores s3 (w3x fp8*64 x x2 fp8)
      att numerator (xO fp8 x eh fp8)   att stored fp8*8
      fuse gates: x-half bf16 (W bf16*256) + att-half fp8 DR (att*8 x W*32)
    All scales undone via scalar.activation(func, scale=2^-k).
  - Attention prep (row-major transposes, s1/s2/thr) is emitted per-slab
    inside highway layer 2, filling PE slack in the elementwise-bound
    highway stretch.
  - Phase D per (b,h) unit is two-staged: (1) all four j-tile-pair score
    blocks -> exp -> fp8 eh pair tiles (SBUF), (2) denominator then
    du-major numerator accumulation, so wide score psum is double-buffered
    within the 8-bank budget.
  - eh pair tiles [128,2,512] fp8 are exactly the DoubleRow moving operand
    of the numerator.  relu-as-clamp: M^T = max(exp(s3+s2), exp(-(s1+ab)))
    (the per-column factor exp(s1+ab) cancels in the softmax).
"""

import numpy as np

B, L, U, H = 16, 1024, 512, 2
NCORES = 8
BPC = B // NCORES          # batches per core
N = BPC * L                # token columns per core
KU = U // 128              # 4  u-tiles
NT = N // 128              # 16 row-tiles per core
NS = N // 512              # 4  512-wide column slices per core
JT = L // 128              # 8  j-tiles per batch
IH = L // 512              # 2  i-halves per batch

WSH = 32.0                 # highway weight prescale (2^5)
WSA = 64.0                 # aW prescale (2^6)
WSF = 256.0                # fuse-gate effective prescale (2^8)
ATS = 8.0                  # att fp8 prescale (2^3)


def build_nc():
    import concourse.bacc as bacc
    import concourse.tile as tile
    from concourse import mybir
    from concourse.masks import make_identity

    F32 = mybir.dt.float32
    BF16 = mybir.dt.bfloat16
    F8 = mybir.dt.float8e4
    AF = mybir.ActivationFunctionType
    OP = mybir.AluOpType
    DR = mybir.MatmulPerfMode.DoubleRow

    nc = bacc.Bacc("TRN2", target_bir_lowering=False, debug=False,
                   num_devices=NCORES)

    x_in = nc.dram_tensor("inputs", [BPC, L, U], F32, kind="ExternalInput").ap()
    tW = nc.dram_tensor("tW", [H, U, U], F32, kind="ExternalInput").ap()
    tb = nc.dram_tensor("tb", [H, U], F32, kind="ExternalInput").ap()
    cW = nc.dram_tensor("cW", [H, U, U], F32, kind="ExternalInput").ap()
    cb = nc.dram_tensor("cb", [H, U], F32, kind="ExternalInput").ap()
    aW = nc.dram_tensor("aW", [3 * U], F32, kind="ExternalInput").ap()
    ab = nc.dram_tensor("ab", [1], F32, kind="ExternalInput").ap()
    frW = nc.dram_tensor("frW", [2 * U, U], F32, kind="ExternalInput").ap()
    frb = nc.dram_tensor("frb", [U], F32, kind="ExternalInput").ap()
    ffW = nc.dram_tensor("ffW", [2 * U, U], F32, kind="ExternalInput").ap()
    ffb = nc.dram_tensor("ffb", [U], F32, kind="ExternalInput").ap()
    out = nc.dram_tensor("out", [BPC, L, U], F32, kind="ExternalOutput").ap()

    xv = x_in.flatten_outer_dims().rearrange("(t p) u -> t p u", p=128)
    outv = out.flatten_outer_dims().rearrange("(t p) u -> t p u", p=128)

    def pair(t, k2, sl=None):
        """[128, 2, *] DoubleRow view of adjacent k-tiles k2*2, k2*2+1."""
        return t[:, 2 * k2:2 * k2 + 2, sl] if sl is not None \
            else t[:, 2 * k2:2 * k2 + 2, :]

    with tile.TileContext(nc) as tc:
        with tc.tile_pool(name="pers", bufs=1) as pers:
            # ---- persistent SBUF tensors ----
            x0row = pers.tile([128, NT, U], F32, tag="x0row")  # inputs row-maj
            xTh = pers.tile([128, KU, N], BF16, tag="xTh")     # inputs^T bf16
            x0q8 = pers.tile([128, KU, N], F8, tag="x0q8")     # inputs^T fp8
            x1q8 = pers.tile([128, KU, N], F8, tag="x1q8")
            x2q8 = pers.tile([128, KU, N], F8, tag="x2q8")
            w3x8 = pers.tile([128, KU, N], F8, tag="w3x8")     # (w3*64)*x2^T
            attT8 = pers.tile([128, KU, N], F8, tag="attT8")   # att^T * 8
            xO8 = pers.tile([128, NT, U], F8, tag="xO8")       # row-major x2
            tWh8 = pers.tile([128, H, KU, U], F8, tag="tWh8")  # *32
            cWh8 = pers.tile([128, H, KU, U], F8, tag="cWh8")  # *32
            ffWx = pers.tile([128, KU, U], BF16, tag="ffWx")   # x-half *256
            frWx = pers.tile([128, KU, U], BF16, tag="frWx")
            ffW8 = pers.tile([128, KU, U], F8, tag="ffW8")     # att-half *32
            frW8 = pers.tile([128, KU, U], F8, tag="frW8")
            tbsb = pers.tile([128, H, KU], F32, tag="tbsb")
            cbsb = pers.tile([128, H, KU], F32, tag="cbsb")
            awsb = pers.tile([128, 12], F32, tag="awsb")       # w1|w2|w3 cols
            w1h8 = pers.tile([128, KU, 16], F8, tag="w1h8")    # *64, col 0
            w2h8 = pers.tile([128, KU, 16], F8, tag="w2h8")    # *64, col 0
            aw3s = pers.tile([128, KU], F32, tag="aw3s")       # w3 * 64 f32
            ab_sb = pers.tile([1, 1], F32, tag="ab_sb")
            nab_sb = pers.tile([1, 1], F32, tag="nab_sb")
            ffb_h = pers.tile([1, U], BF16, tag="ffb_h")       # *256
            frb_h = pers.tile([1, U], BF16, tag="frb_h")       # *256
            thr = pers.tile([1, N], BF16, tag="thr")   # exp(-(s1+ab))
            s2f = pers.tile([128, NT], F32, tag="s2f")
            ones_row = pers.tile([1, 128], BF16, tag="ones_row")
            ones2c8 = pers.tile([128, 32], F8, tag="ones2c8")  # DR ones pairs
            identb = pers.tile([128, 128], BF16, tag="identb")
            ident8 = pers.tile([128, 128], F8, tag="ident8")
            identf = pers.tile([128, 128], F32, tag="identf")

            nc.vector.memset(ones_row, 1.0)
            nc.vector.memset(ones2c8, 1.0)
            make_identity(nc, identb)
            make_identity(nc, ident8)
            make_identity(nc, identf)

            # ================= Phase A: loads, casts, input transpose ======
            with tc.tile_pool(name="stg", bufs=8) as stg, \
                 tc.tile_pool(name="stgw", bufs=8) as stgw, \
                 tc.tile_pool(name="ptA", bufs=1, space="PSUM") as ptA:
                warmp = ptA.tile([128, 512], F32, tag="warmp")

                def keep_warm(n, who):
                    for i in range(n):
                        nc.tensor.matmul(warmp[:, 0:128], identb, identb,
                                         start=True, stop=True)

                # highway-weight loads interleaved after tg0/tg1 so layer-0
                # can start as soon as the first column group lands; one
                # 1MB DMA + one wide cast per (layer, gate)
                def emit_weights(l, wi):
                    wsrc, wdst = ((tW, tWh8), (cW, cWh8))[wi]
                    wv = wsrc[l].rearrange("(k p) m -> p k m", p=128)
                    ws = stgw.tile([128, KU, U], F32, tag="ws",
                                   name=f"ws_{l}_{wi}")
                    nc.sync.dma_start(ws, wv)
                    if wi == 0:
                        nc.vector.tensor_scalar_mul(wdst[:, l], ws, WSH)
                    else:
                        nc.scalar.mul(wdst[:, l], ws, WSH)

                # warm the PE HAM clock-gate during the initial DMA wait
                keep_warm(48, "init")
                for tg in range(NS):
                    # one 1MB DMA per 512-token group, straight into the
                    # persistent row-major copy (reused by phase E)
                    nc.sync.dma_start(
                        x0row[:, 4 * tg:4 * tg + 4, :],
                        x_in.flatten_outer_dims().rearrange(
                            "(t p) u -> p t u", p=128)[:, 4 * tg:4 * tg + 4])
                    ptk = [ptA.tile([128, 512], F32, tag=f"ptk{k}",
                                    name=f"ptk_{tg}_{k}")
                           for k in range(KU)]
                    for tt in range(4):
                        t = tg * 4 + tt
                        for k in range(KU):
                            nc.tensor.transpose(
                                ptk[k][:, tt * 128:(tt + 1) * 128],
                                x0row[:, t, k * 128:(k + 1) * 128], identf)
                    for k in range(KU):
                        sl = slice(tg * 512, (tg + 1) * 512)
                        if k % 2 == 0:
                            nc.vector.tensor_copy(xTh[:, k, sl], ptk[k])
                            nc.scalar.copy(x0q8[:, k, sl], ptk[k])
                        else:
                            nc.scalar.copy(xTh[:, k, sl], ptk[k])
                            nc.vector.tensor_copy(x0q8[:, k, sl], ptk[k])
                    keep_warm(10, f"tg{tg}")
                    if tg < H:
                        emit_weights(0, tg)
                    elif tg == H:
                        nc.sync.dma_start(
                            tbsb, tb.rearrange("l (m p) -> p l m", p=128))
                        nc.sync.dma_start(
                            cbsb, cb.rearrange("l (m p) -> p l m", p=128))
                        nc.sync.dma_start(
                            awsb, aW.rearrange("(w m p) -> p (w m)",
                                               p=128, w=3))
                        for k in range(KU):
                            nc.vector.tensor_scalar_mul(
                                w1h8[:, k, 0:1], awsb[:, k:k + 1], WSA)
                            nc.vector.tensor_scalar_mul(
                                w2h8[:, k, 0:1], awsb[:, KU + k:KU + k + 1],
                                WSA)
                            nc.scalar.mul(aw3s[:, k:k + 1],
                                          awsb[:, 8 + k:9 + k], WSA)
                        nc.sync.dma_start(ab_sb, ab[None, :])
                        nc.scalar.mul(nab_sb, ab_sb, -1.0)
                        fb = stg.tile([1, U], F32, tag="fb")
                        nc.sync.dma_start(fb, ffb[None, :])
                        nc.vector.tensor_scalar_mul(ffb_h, fb, WSF)
                        fb2 = stg.tile([1, U], F32, tag="fb")
                        nc.sync.dma_start(fb2, frb[None, :])
                        nc.vector.tensor_scalar_mul(frb_h, fb2, WSF)
                    else:
                        emit_weights(1, 0)
                        emit_weights(1, 1)

            # ===== Phase B+C: slab-interleaved highway + attention prep ====
            # l0/l1 iterations are interleaved per 512-token slab so l1
            # matmuls fill PE while l0's elementwise chain drains, and each
            # finished slab-pair immediately gets its row-major transposes,
            # w3x, s1/thr and s2 emitted.
            with tc.tile_pool(name="hwp", bufs=2, space="PSUM") as hwp, \
                 tc.tile_pool(name="pcp", bufs=2, space="PSUM") as pcp, \
                 tc.tile_pool(name="pcp1", bufs=1, space="PSUM") as pcp1, \
                 tc.tile_pool(name="hws", bufs=3) as hws:
                s2p = pcp1.tile([128, NT], F32, tag="s2p")

                def prep_slab(tp):
                    """attention prep for tokens [tp*1024, (tp+1)*1024)."""
                    for k in range(KU):
                        wsl = slice(tp * 1024, (tp + 1) * 1024)
                        nc.vector.tensor_scalar_mul(
                            w3x8[:, k, wsl], x2q8[:, k, wsl],
                            aw3s[:, k:k + 1])
                    for jt in range(8 * tp, 8 * tp + 8):
                        # full-bank staging tile so rotating bufs land in
                        # different banks (PE-write vs DVE-read collision)
                        ptr = pcp.tile([128, 2048], F8, tag="ptr")
                        ptv = ptr[:, 0:1024].rearrange(
                            "p (n two) -> p n two", two=2)
                        for k in range(KU):
                            nc.tensor.transpose(
                                ptv[:, k * 128:(k + 1) * 128, 0:1],
                                x2q8[:, k, jt * 128:(jt + 1) * 128], ident8)
                        if jt % 2 == 0:
                            nc.vector.tensor_copy(xO8[:, jt, :],
                                                  ptv[:, :, 0:1])
                        else:
                            nc.scalar.copy(xO8[:, jt, :], ptv[:, :, 0:1])
                    for t in (2 * tp, 2 * tp + 1):
                        ps1 = pcp1.tile([1, 512], F32, tag="ps1")
                        for kk in range(KU // 2):
                            nc.tensor.matmul(
                                ps1, pair(w1h8, kk, slice(0, 1)),
                                pair(x2q8, kk,
                                     slice(t * 512, (t + 1) * 512)),
                                perf_mode=DR,
                                start=(kk == 0), stop=(kk == 1))
                        nc.scalar.activation(
                            thr[:, t * 512:(t + 1) * 512], ps1, AF.Exp,
                            bias=nab_sb, scale=-1.0 / WSA)
                    for jt in range(8 * tp, 8 * tp + 8):
                        jsl = slice(jt * 128, (jt + 1) * 128)
                        for kk in range(KU // 2):
                            nc.tensor.matmul(
                                s2p[:, jt:jt + 1],
                                pair(x2q8, kk, jsl),
                                pair(w2h8, kk, slice(0, 1)),
                                perf_mode=DR,
                                start=(kk == 0), stop=(kk == 1))
                    nc.scalar.mul(s2f[:, 8 * tp:8 * tp + 8],
                                  s2p[:, 8 * tp:8 * tp + 8], 1.0 / WSA)

                def hw_slab(l, t):
                    xin = x0q8 if l == 0 else x1q8
                    xout = x1q8 if l == 0 else x2q8
                    nsl = slice(t * 512, (t + 1) * 512)
                    for m in range(KU):
                        msl = slice(m * 128, (m + 1) * 128)
                        pt = hwp.tile([128, 512], F32, tag="pt")
                        pc = hwp.tile([128, 512], F32, tag="pc")
                        for kk in range(KU // 2):
                            nc.tensor.matmul(
                                pt, pair(tWh8[:, l], kk, msl),
                                pair(xin, kk, nsl), perf_mode=DR,
                                start=(kk == 0), stop=(kk == 1))
                        for kk in range(KU // 2):
                            nc.tensor.matmul(
                                pc, pair(cWh8[:, l], kk, msl),
                                pair(xin, kk, nsl), perf_mode=DR,
                                start=(kk == 0), stop=(kk == 1))
                        th = hws.tile([128, 512], BF16, tag="th")
                        ch = hws.tile([128, 512], BF16, tag="ch")
                        nc.scalar.activation(
                            th, pt, AF.Relu, bias=tbsb[:, l, m:m + 1],
                            scale=1.0 / WSH)
                        nc.scalar.activation(
                            ch, pc, AF.Sigmoid, bias=cbsb[:, l, m:m + 1],
                            scale=1.0 / WSH)
                        dh = hws.tile([128, 512], BF16, tag="dh")
                        nc.vector.tensor_tensor(
                            dh, th, xin[:, m, nsl], op=OP.subtract)
                        mh = hws.tile([128, 512], BF16, tag="mh")
                        nc.vector.tensor_tensor(
                            mh, ch, dh, op=OP.mult)
                        nc.gpsimd.tensor_tensor(
                            xout[:, m, nsl], xin[:, m, nsl], mh,
                            op=OP.add)

                hw_slab(0, 0)
                hw_slab(0, 1)
                hw_slab(1, 0)
                hw_slab(0, 2)
                hw_slab(1, 1)
                hw_slab(0, 3)
                hw_slab(1, 2)
                prep_slab(0)
                hw_slab(1, 3)
                prep_slab(1)

            # ============= Phase D: pairwise softmax attention =============
            fWv = ffW.rearrange("(k p) m -> k p m", p=128)
            rWv = frW.rearrange("(k p) m -> k p m", p=128)
            # x-half (k 0..3) -> bf16 *256 ; att-half (k 4..7) -> fp8 *32
            fuse_chunks = ([(fWv, ffWx, ffW8, k) for k in range(2 * KU)] +
                           [(rWv, frWx, frW8, k) for k in range(2 * KU)])
            with tc.tile_pool(name="pdn", bufs=2, space="PSUM") as pdn, \
                 tc.tile_pool(name="pds", bufs=2, space="PSUM") as pds, \
                 tc.tile_pool(name="pdr", bufs=1, space="PSUM") as pdr, \
                 tc.tile_pool(name="pbc", bufs=1, space="PSUM") as pbc, \
                 tc.tile_pool(name="stgf", bufs=3) as stgf, \
                 tc.tile_pool(name="dsb", bufs=3) as dsb, \
                 tc.tile_pool(name="ehp", bufs=6) as ehp:
                ones2v = ones2c8.rearrange("p (two s) -> p two s", two=2)
                # fuse-gate weight loads up front: one DMA burst, casts
                # overlap the first attention units on vector/scalar
                for ci in range(len(fuse_chunks)):
                    wv_, wbf_, w8_, k_ = fuse_chunks[ci]
                    wsf = stgf.tile([128, U], F32, tag="wsf",
                                    name=f"wsf_{ci}")
                    nc.sync.dma_start(wsf, wv_[k_])
                    if k_ < KU:
                        if ci % 2 == 0:
                            nc.vector.tensor_scalar_mul(
                                wbf_[:, k_, :], wsf, WSF)
                        else:
                            nc.scalar.mul(wbf_[:, k_, :], wsf, WSF)
                    else:
                        if ci % 2 == 0:
                            nc.vector.tensor_scalar_mul(
                                w8_[:, k_ - KU, :], wsf, WSH)
                        else:
                            nc.scalar.mul(w8_[:, k_ - KU, :], wsf, WSH)
                for b in range(BPC):
                    for h in range(IH):
                        isl = slice(b * L + h * 512, b * L + (h + 1) * 512)
                        thbc = dsb.tile([128, 512], BF16, tag="thbc")
                        pb1 = pbc.tile([128, 512], F32, tag="pb",
                                       name=f"pb1_{b}_{h}")
                        nc.tensor.matmul(pb1, ones_row, thr[:, isl],
                                         start=True, stop=True)
                        nc.scalar.copy(thbc, pb1)
                        # ---- stage 1: scores -> exp -> fp8 eh pair tiles
                        ehs = []
                        for p in range(JT // 2):      # j-tile pairs
                            jg = b * JT + 2 * p
                            ps = pds.tile([128, 1024], F32, tag="ps",
                                          name=f"ps_{b}_{h}_{p}")
                            ehb = ehp.tile([128, 1024], F8, tag="ehb",
                                           name=f"ehb_{b}_{h}_{p}")
                            ehbf = dsb.tile([128, 1024], BF16, tag="ehbf")
                            for half in range(2):
                                jsl = slice((jg + half) * 128,
                                            (jg + half + 1) * 128)
                                hsl = slice(half * 512, (half + 1) * 512)
                                for kk in range(KU // 2):
                                    nc.tensor.matmul(
                                        ps[:, hsl], pair(w3x8, kk, jsl),
                                        pair(x2q8, kk, isl), perf_mode=DR,
                                        start=(kk == 0), stop=(kk == 1))
                                nc.scalar.activation(
                                    ehbf[:, hsl], ps[:, hsl], AF.Exp,
                                    bias=s2f[:, jg + half:jg + half + 1],
                                    scale=1.0 / WSA)
                                nc.vector.tensor_tensor(
                                    ehb[:, hsl], ehbf[:, hsl], thbc,
                                    op=OP.max)
                            ehs.append(
                                ehb.rearrange("p (two n) -> p two n", two=2))
                        # ---- stage 2: denominator first, then du-major
                        # numerator accumulation
                        pr = pdr.tile([1, 512], F32, tag="pr")
                        for p in range(JT // 2):
                            nc.tensor.matmul(
                                pr, ones2v[:, :, 0:1], ehs[p], perf_mode=DR,
                                start=(p == 0), stop=(p == JT // 2 - 1))
                        rec = dsb.tile([1, 512], F32, tag="rec")
                        nc.vector.reciprocal_approx_fast(rec, pr)
                        rech = dsb.tile([1, 512], BF16, tag="rech")
                        nc.scalar.mul(rech, rec, ATS)
                        rbc = dsb.tile([128, 512], BF16, tag="rbc")
                        pb2 = pbc.tile([128, 512], F32, tag="pb",
                                       name=f"pb2_{b}_{h}")
                        nc.tensor.matmul(pb2, ones_row, rech,
                                         start=True, stop=True)
                        nc.scalar.copy(rbc, pb2)
                        for du in range(KU):
                            pn = pdn.tile([128, 512], F32, tag="pn",
                                          name=f"pn_{b}_{h}_{du}")
                            for p in range(JT // 2):
                                jg = b * JT + 2 * p
                                nc.tensor.matmul(
                                    pn,
                                    xO8[:, jg:jg + 2,
                                        du * 128:(du + 1) * 128],
                                    ehs[p], perf_mode=DR,
                                    start=(p == 0), stop=(p == JT // 2 - 1))
                            # drain + normalize (*8) in one pass
                            nc.vector.tensor_tensor(
                                attT8[:, du, isl], pn, rbc, op=OP.mult)

            # ============= Phase E: fuse gates + output ====================
            with tc.tile_pool(name="pep", bufs=2, space="PSUM") as pep, \
                 tc.tile_pool(name="peb", bufs=1, space="PSUM") as peb, \
                 tc.tile_pool(name="esb", bufs=3) as esb:
                # broadcast fuse biases (*256) to [128, 512] once
                fbb = esb.tile([128, U], BF16, tag="fbb")
                rbb = esb.tile([128, U], BF16, tag="rbb")
                pfb = peb.tile([128, 512], F32, tag="pfb", name="pfb_f")
                nc.tensor.matmul(pfb, ones_row, ffb_h, start=True, stop=True)
                nc.vector.tensor_copy(fbb, pfb)
                prb = peb.tile([128, 512], F32, tag="pfb", name="pfb_r")
                nc.tensor.matmul(prb, ones_row, frb_h, start=True, stop=True)
                nc.vector.tensor_copy(rbb, prb)
                for mt in range(NT):
                    msl = slice(mt * 128, (mt + 1) * 128)
                    pz = pep.tile([128, 512], F32, tag="pz")
                    pr2 = pep.tile([128, 512], F32, tag="pr2")
                    for k in range(KU):          # x-half, bf16
                        nc.tensor.matmul(pz, xTh[:, k, msl], ffWx[:, k, :],
                                         start=(k == 0), stop=False)
                        nc.tensor.matmul(pr2, xTh[:, k, msl], frWx[:, k, :],
                                         start=(k == 0), stop=False)
                    for kk in range(KU // 2):    # att-half, fp8 DR
                        nc.tensor.matmul(pz, pair(attT8, kk, msl),
                                         pair(ffW8, kk), perf_mode=DR,
                                         start=False, stop=(kk == 1))
                        nc.tensor.matmul(pr2, pair(attT8, kk, msl),
                                         pair(frW8, kk), perf_mode=DR,
                                         start=False, stop=(kk == 1))
                    # bias add on vector (frees psum early), sigmoid on
                    # scalar from SBUF
                    pzs = esb.tile([128, U], BF16, tag="pzs")
                    prs = esb.tile([128, U], BF16, tag="prs")
                    nc.vector.tensor_tensor(pzs, pz, fbb, op=OP.add)
                    nc.vector.tensor_tensor(prs, pr2, rbb, op=OP.add)
                    zh = esb.tile([128, U], BF16, tag="zh")
                    rh = esb.tile([128, U], BF16, tag="rh")
                    q = esb.tile([128, U], F32, tag="q")
                    p2 = esb.tile([128, U], F32, tag="p2")
                    ot = esb.tile([128, U], F32, tag="ot")
                    x0t = x0row[:, mt, :]
                    if mt == NT - 1:
                        # shorten the kernel tail: split across engines
                        hU = U // 2
                        nc.scalar.activation(zh, pzs, AF.Sigmoid,
                                             scale=1.0 / WSF)
                        nc.scalar.square(q, zh)
                        nc.scalar.activation(rh, prs, AF.Sigmoid,
                                             scale=1.0 / WSF)
                        nc.vector.tensor_tensor(p2[:, :hU], rh[:, :hU],
                                                x0t[:, :hU], op=OP.mult)
                        nc.gpsimd.tensor_tensor(p2[:, hU:], rh[:, hU:],
                                                x0t[:, hU:], op=OP.mult)
                        nc.vector.tensor_tensor(ot[:, :hU], q[:, :hU],
                                                p2[:, :hU], op=OP.add)
                        nc.gpsimd.tensor_tensor(ot[:, hU:], q[:, hU:],
                                                p2[:, hU:], op=OP.add)
                    else:
                        nc.scalar.activation(zh, pzs, AF.Sigmoid,
                                             scale=1.0 / WSF)
                        nc.scalar.activation(rh, prs, AF.Sigmoid,
                                             scale=1.0 / WSF)
                        nc.scalar.square(q, zh)
                        nc.vector.tensor_tensor(p2, rh, x0t, op=OP.mult)
                        nc.vector.tensor_tensor(ot, q, p2, op=OP.add)
                    nc.sync.dma_start(outv[mt], ot)

    nc.compile()
    return nc


_NC_CACHE = None


def _get_nc():
    global _NC_CACHE
    if _NC_CACHE is None:
        _NC_CACHE = build_nc()
    return _NC_CACHE


def kernel(**inputs) -> np.ndarray:
    from concourse.bass_utils import run_bass_kernel_spmd

    nc = _get_nc()
    full = {k: np.ascontiguousarray(np.asarray(v, dtype=np.float32))
            for k, v in inputs.items()}
    in_maps = []
    for c in range(NCORES):
        m = dict(full)
        m["inputs"] = np.ascontiguousarray(
            full["inputs"][c * BPC:(c + 1) * BPC])
        in_maps.append(m)
    res = run_bass_kernel_spmd(nc, in_maps, core_ids=list(range(NCORES)))
    return np.concatenate([res.results[c]["out"] for c in range(NCORES)],
                          axis=0)


# revision 30
# speedup vs baseline: 1.0373x; 1.0373x over previous
"""Trainium2 Bass kernel for nn_Encoding_layer (highway stack + pairwise MLP
attention + fuse gates).

Sharding: data-parallel over batch B=16 across 8 NeuronCores (2 batches per
core); all dense weights replicated. No collectives.

v3: fp8-e4m3 DoubleRow matmuls for the compute-heavy GEMMs, with the
schedule restructured to keep the PE HAM clock-gate warm:
  - DoubleRow contracts 256 rows/pass (2 fp8 weights per PE cell); operand
    pairs are adjacent k-tiles in the free dim of the [128, KU, N] tilings.
  - Quantization (numpy-validated, rel err ~3e-3 vs 2e-2 budget):
      highway (x fp8, W fp8*32)   scores s3 (w3x fp8*64 x x2 fp8)
      att numerator (xO fp8 x eh fp8)   att stored fp8*8
      fuse gates: x-half bf16 (W bf16*256) + att-half fp8 DR (att*8 x W*32)
    All scales undone via scalar.activation(func, scale=2^-k).
  - Attention prep (row-major transposes, s1/s2/thr) is emitted per-slab
    inside highway layer 2, filling PE slack in the elementwise-bound
    highway stretch.
  - Phase D per (b,h) unit is two-staged: (1) all four j-tile-pair score
    blocks -> exp -> fp8 eh pair tiles (SBUF), (2) denominator then
    du-major numerator accumulation, so wide score psum is double-buffered
    within the 8-bank budget.
  - eh pair tiles [128,2,512] fp8 are exactly the DoubleRow moving operand
    of the numerator.  relu-as-clamp: M^T = max(exp(s3+s2), exp(-(s1+ab)))
    (the per-column factor exp(s1+ab) cancels in the softmax).
"""

import numpy as np

B, L, U, H = 16, 1024, 512, 2
NCORES = 8
BPC = B // NCORES          # batches per core
N = BPC * L                # token columns per core
KU = U // 128              # 4  u-tiles
NT = N // 128              # 16 row-tiles per core
NS = N // 512              # 4  512-wide column slices per core
JT = L // 128              # 8  j-tiles per batch
IH = L // 512              # 2  i-halves per batch

WSH = 32.0                 # highway weight prescale (2^5)
WSA = 64.0                 # aW prescale (2^6)
WSF = 256.0                # fuse-gate effective prescale (2^8)
ATS = 8.0                  # att fp8 prescale (2^3)


def build_nc():
    import concourse.bacc as bacc
    import concourse.tile as tile
    from concourse import mybir
    from concourse.masks import make_identity

    F32 = mybir.dt.float32
    BF16 = mybir.dt.bfloat16
    F8 = mybir.dt.float8e4
    AF = mybir.ActivationFunctionType
    OP = mybir.AluOpType
    DR = mybir.MatmulPerfMode.DoubleRow

    nc = bacc.Bacc("TRN2", target_bir_lowering=False, debug=False,
                   num_devices=NCORES)

    x_in = nc.dram_tensor("inputs", [BPC, L, U], F32, kind="ExternalInput").ap()
    tW = nc.dram_tensor("tW", [H, U, U], F32, kind="ExternalInput").ap()
    tb = nc.dram_tensor("tb", [H, U], F32, kind="ExternalInput").ap()
    cW = nc.dram_tensor("cW", [H, U, U], F32, kind="ExternalInput").ap()
    cb = nc.dram_tensor("cb", [H, U], F32, kind="ExternalInput").ap()
    aW = nc.dram_tensor("aW", [3 * U], F32, kind="ExternalInput").ap()
    ab = nc.dram_tensor("ab", [1], F32, kind="ExternalInput").ap()
    frW = nc.dram_tensor("frW", [2 * U, U], F32, kind="ExternalInput").ap()
    frb = nc.dram_tensor("frb", [U], F32, kind="ExternalInput").ap()
    ffW = nc.dram_tensor("ffW", [2 * U, U], F32, kind="ExternalInput").ap()
    ffb = nc.dram_tensor("ffb", [U], F32, kind="ExternalInput").ap()
    out = nc.dram_tensor("out", [BPC, L, U], F32, kind="ExternalOutput").ap()

    xv = x_in.flatten_outer_dims().rearrange("(t p) u -> t p u", p=128)
    outv = out.flatten_outer_dims().rearrange("(t p) u -> t p u", p=128)

    def pair(t, k2, sl=None):
        """[128, 2, *] DoubleRow view of adjacent k-tiles k2*2, k2*2+1."""
        return t[:, 2 * k2:2 * k2 + 2, sl] if sl is not None \
            else t[:, 2 * k2:2 * k2 + 2, :]

    with tile.TileContext(nc) as tc:
        with tc.tile_pool(name="pers", bufs=1) as pers:
            # ---- persistent SBUF tensors ----
            x0row = pers.tile([128, NT, U], F32, tag="x0row")  # inputs row-maj
            xTh = pers.tile([128, KU, N], BF16, tag="xTh")     # inputs^T bf16
            x0q8 = pers.tile([128, KU, N], F8, tag="x0q8")     # inputs^T fp8
            x1q8 = pers.tile([128, KU, N], F8, tag="x1q8")
            x2q8 = pers.tile([128, KU, N], F8, tag="x2q8")
            w3x8 = pers.tile([128, KU, N], F8, tag="w3x8")     # (w3*64)*x2^T
            attT8 = pers.tile([128, KU, N], F8, tag="attT8")   # att^T * 8
            xO8 = pers.tile([128, NT, U], F8, tag="xO8")       # row-major x2
            tWh8 = pers.tile([128, H, KU, U], F8, tag="tWh8")  # *32
            cWh8 = pers.tile([128, H, KU, U], F8, tag="cWh8")  # *32
            ffWx = pers.tile([128, KU, U], BF16, tag="ffWx")   # x-half *256
            frWx = pers.tile([128, KU, U], BF16, tag="frWx")
            ffW8 = pers.tile([128, KU, U], F8, tag="ffW8")     # att-half *32
            frW8 = pers.tile([128, KU, U], F8, tag="frW8")
            tbsb = pers.tile([128, H, KU], F32, tag="tbsb")
            cbsb = pers.tile([128, H, KU], F32, tag="cbsb")
            awsb = pers.tile([128, 12], F32, tag="awsb")       # w1|w2|w3 cols
            w1h8 = pers.tile([128, KU, 16], F8, tag="w1h8")    # *64, col 0
            w2h8 = pers.tile([128, KU, 16], F8, tag="w2h8")    # *64, col 0
            aw3s = pers.tile([128, KU], F32, tag="aw3s")       # w3 * 64 f32
            ab_sb = pers.tile([1, 1], F32, tag="ab_sb")
            nab_sb = pers.tile([1, 1], F32, tag="nab_sb")
            ffb_h = pers.tile([1, U], BF16, tag="ffb_h")       # *256
            frb_h = pers.tile([1, U], BF16, tag="frb_h")       # *256
            thr = pers.tile([1, N], BF16, tag="thr")   # exp(-(s1+ab))
            s2f = pers.tile([128, NT], F32, tag="s2f")
            ones_row = pers.tile([1, 128], BF16, tag="ones_row")
            ones2c8 = pers.tile([128, 32], F8, tag="ones2c8")  # DR ones pairs
            identb = pers.tile([128, 128], BF16, tag="identb")
            ident8 = pers.tile([128, 128], F8, tag="ident8")
            identf = pers.tile([128, 128], F32, tag="identf")

            nc.vector.memset(ones_row, 1.0)
            nc.vector.memset(ones2c8, 1.0)
            make_identity(nc, identb)
            make_identity(nc, ident8)
            make_identity(nc, identf)

            # ================= Phase A: loads, casts, input transpose ======
            with tc.tile_pool(name="stg", bufs=8) as stg, \
                 tc.tile_pool(name="stgw", bufs=8) as stgw, \
                 tc.tile_pool(name="ptA", bufs=1, space="PSUM") as ptA:
                warmp = ptA.tile([128, 512], F32, tag="warmp")

                def keep_warm(n, who):
                    for i in range(n):
                        nc.tensor.matmul(warmp[:, 0:128], identb, identb,
                                         start=True, stop=True)

                # highway-weight loads interleaved after tg0/tg1 so layer-0
                # can start as soon as the first column group lands; one
                # 1MB DMA + one wide cast per (layer, gate)
                def emit_weights(l, wi):
                    wsrc, wdst = ((tW, tWh8), (cW, cWh8))[wi]
                    wv = wsrc[l].rearrange("(k p) m -> p k m", p=128)
                    ws = stgw.tile([128, KU, U], F32, tag="ws",
                                   name=f"ws_{l}_{wi}")
                    nc.sync.dma_start(ws, wv)
                    if wi == 0:
                        nc.vector.tensor_scalar_mul(wdst[:, l], ws, WSH)
                    else:
                        nc.scalar.mul(wdst[:, l], ws, WSH)

                # warm the PE HAM clock-gate during the initial DMA wait
                keep_warm(48, "init")
                for tg in range(NS):
                    # one 1MB DMA per 512-token group, straight into the
                    # persistent row-major copy (reused by phase E)
                    nc.sync.dma_start(
                        x0row[:, 4 * tg:4 * tg + 4, :],
                        x_in.flatten_outer_dims().rearrange(
                            "(t p) u -> p t u", p=128)[:, 4 * tg:4 * tg + 4])
                    ptk = [ptA.tile([128, 512], F32, tag=f"ptk{k}",
                                    name=f"ptk_{tg}_{k}")
                           for k in range(KU)]
                    for tt in range(4):
                        t = tg * 4 + tt
                        for k in range(KU):
                            nc.tensor.transpose(
                                ptk[k][:, tt * 128:(tt + 1) * 128],
                                x0row[:, t, k * 128:(k + 1) * 128], identf)
                    for k in range(KU):
                        sl = slice(tg * 512, (tg + 1) * 512)
                        if k % 2 == 0:
                            nc.vector.tensor_copy(xTh[:, k, sl], ptk[k])
                            nc.scalar.copy(x0q8[:, k, sl], ptk[k])
                        else:
                            nc.scalar.copy(xTh[:, k, sl], ptk[k])
                            nc.vector.tensor_copy(x0q8[:, k, sl], ptk[k])
                    keep_warm(10, f"tg{tg}")
                    if tg < H:
                        emit_weights(0, tg)
                    elif tg == H:
                        nc.sync.dma_start(
                            tbsb, tb.rearrange("l (m p) -> p l m", p=128))
                        nc.sync.dma_start(
                            cbsb, cb.rearrange("l (m p) -> p l m", p=128))
                        nc.sync.dma_start(
                            awsb, aW.rearrange("(w m p) -> p (w m)",
                                               p=128, w=3))
                        for k in range(KU):
                            nc.vector.tensor_scalar_mul(
                                w1h8[:, k, 0:1], awsb[:, k:k + 1], WSA)
                            nc.vector.tensor_scalar_mul(
                                w2h8[:, k, 0:1], awsb[:, KU + k:KU + k + 1],
                                WSA)
                            nc.scalar.mul(aw3s[:, k:k + 1],
                                          awsb[:, 8 + k:9 + k], WSA)
                        nc.sync.dma_start(ab_sb, ab[None, :])
                        nc.scalar.mul(nab_sb, ab_sb, -1.0)
                        fb = stg.tile([1, U], F32, tag="fb")
                        nc.sync.dma_start(fb, ffb[None, :])
                        nc.vector.tensor_scalar_mul(ffb_h, fb, WSF)
                        fb2 = stg.tile([1, U], F32, tag="fb")
                        nc.sync.dma_start(fb2, frb[None, :])
                        nc.vector.tensor_scalar_mul(frb_h, fb2, WSF)
                    else:
                        emit_weights(1, 0)
                        emit_weights(1, 1)

            # ===== Phase B layer 0: wide [128,1024] 2-bank psum tiles ======
            with tc.tile_pool(name="hw0", bufs=2, space="PSUM") as hw0, \
                 tc.tile_pool(name="hs0", bufs=3) as hs0:
                for tp in range(NS // 2):              # 1024-token slabs
                    wsl = slice(tp * 1024, (tp + 1) * 1024)
                    for m in range(KU):
                        msl = slice(m * 128, (m + 1) * 128)
                        pt = hw0.tile([128, 1024], F32, tag="pt")
                        pc = hw0.tile([128, 1024], F32, tag="pc")
                        for h2 in range(2):
                            nsl = slice(tp * 1024 + h2 * 512,
                                        tp * 1024 + (h2 + 1) * 512)
                            psl = slice(h2 * 512, (h2 + 1) * 512)
                            for kk in range(KU // 2):
                                nc.tensor.matmul(
                                    pt[:, psl], pair(tWh8[:, 0], kk, msl),
                                    pair(x0q8, kk, nsl), perf_mode=DR,
                                    start=(kk == 0), stop=(kk == 1))
                            for kk in range(KU // 2):
                                nc.tensor.matmul(
                                    pc[:, psl], pair(cWh8[:, 0], kk, msl),
                                    pair(x0q8, kk, nsl), perf_mode=DR,
                                    start=(kk == 0), stop=(kk == 1))
                        th = hs0.tile([128, 1024], BF16, tag="th")
                        ch = hs0.tile([128, 1024], BF16, tag="ch")
                        nc.scalar.activation(
                            th, pt, AF.Relu, bias=tbsb[:, 0, m:m + 1],
                            scale=1.0 / WSH)
                        nc.scalar.activation(
                            ch, pc, AF.Sigmoid, bias=cbsb[:, 0, m:m + 1],
                            scale=1.0 / WSH)
                        dh = hs0.tile([128, 1024], BF16, tag="dh")
                        nc.vector.tensor_tensor(
                            dh, th, x0q8[:, m, wsl], op=OP.subtract)
                        mh = hs0.tile([128, 1024], BF16, tag="mh")
                        nc.vector.tensor_tensor(mh, ch, dh, op=OP.mult)
                        nc.gpsimd.tensor_tensor(
                            x1q8[:, m, wsl], x0q8[:, m, wsl], mh, op=OP.add)

            # ===== Phase B layer 1 + C: highway + per-slab attention prep ==
            # During layer 1, each finished 1024-token slab immediately gets
            # its row-major transposes, w3x, s1/thr and s2 emitted, filling
            # PE slack in the elementwise-bound highway stretch.
            with tc.tile_pool(name="hwp", bufs=2, space="PSUM") as hwp, \
                 tc.tile_pool(name="pcp", bufs=2, space="PSUM") as pcp, \
                 tc.tile_pool(name="pcp1", bufs=1, space="PSUM") as pcp1, \
                 tc.tile_pool(name="hws", bufs=3) as hws:
                s2p = pcp1.tile([128, NT], F32, tag="s2p")

                def prep_slab(tp):
                    """attention prep for tokens [tp*1024, (tp+1)*1024)."""
                    for k in range(KU):
                        wsl = slice(tp * 1024, (tp + 1) * 1024)
                        nc.vector.tensor_scalar_mul(
                            w3x8[:, k, wsl], x2q8[:, k, wsl],
                            aw3s[:, k:k + 1])
                    for jt in range(8 * tp, 8 * tp + 8):
                        # full-bank staging tile so rotating bufs land in
                        # different banks (PE-write vs DVE-read collision)
                        ptr = pcp.tile([128, 2048], F8, tag="ptr")
                        ptv = ptr[:, 0:1024].rearrange(
                            "p (n two) -> p n two", two=2)
                        for k in range(KU):
                            nc.tensor.transpose(
                                ptv[:, k * 128:(k + 1) * 128, 0:1],
                                x2q8[:, k, jt * 128:(jt + 1) * 128], ident8)
                        if jt % 2 == 0:
                            nc.vector.tensor_copy(xO8[:, jt, :],
                                                  ptv[:, :, 0:1])
                        else:
                            nc.scalar.copy(xO8[:, jt, :], ptv[:, :, 0:1])
                    for t in (2 * tp, 2 * tp + 1):
                        ps1 = pcp1.tile([1, 512], F32, tag="ps1")
                        for kk in range(KU // 2):
                            nc.tensor.matmul(
                                ps1, pair(w1h8, kk, slice(0, 1)),
                                pair(x2q8, kk,
                                     slice(t * 512, (t + 1) * 512)),
                                perf_mode=DR,
                                start=(kk == 0), stop=(kk == 1))
                        nc.scalar.activation(
                            thr[:, t * 512:(t + 1) * 512], ps1, AF.Exp,
                            bias=nab_sb, scale=-1.0 / WSA)
                    for jt in range(8 * tp, 8 * tp + 8):
                        jsl = slice(jt * 128, (jt + 1) * 128)
                        for kk in range(KU // 2):
                            nc.tensor.matmul(
                                s2p[:, jt:jt + 1],
                                pair(x2q8, kk, jsl),
                                pair(w2h8, kk, slice(0, 1)),
                                perf_mode=DR,
                                start=(kk == 0), stop=(kk == 1))
                    nc.scalar.mul(s2f[:, 8 * tp:8 * tp + 8],
                                  s2p[:, 8 * tp:8 * tp + 8], 1.0 / WSA)

                for t in range(NS):                    # 512-token slabs
                    nsl = slice(t * 512, (t + 1) * 512)
                    for m in range(KU):
                        if t == 0:
                            # keep the PE HAM clock-gate warm through the
                            # l0-tail / l1-ramp valley (prep pool is idle)
                            wt = pcp.tile([128, 2048], F8, tag="ptr",
                                          name=f"wl1_{m}").bitcast(F32)
                            for i in range(4):
                                nc.tensor.matmul(wt[:, 0:128], identb,
                                                 identb, start=True,
                                                 stop=True)
                        msl = slice(m * 128, (m + 1) * 128)
                        pt = hwp.tile([128, 512], F32, tag="pt")
                        pc = hwp.tile([128, 512], F32, tag="pc")
                        for kk in range(KU // 2):
                            nc.tensor.matmul(
                                pt, pair(tWh8[:, 1], kk, msl),
                                pair(x1q8, kk, nsl), perf_mode=DR,
                                start=(kk == 0), stop=(kk == 1))
                        for kk in range(KU // 2):
                            nc.tensor.matmul(
                                pc, pair(cWh8[:, 1], kk, msl),
                                pair(x1q8, kk, nsl), perf_mode=DR,
                                start=(kk == 0), stop=(kk == 1))
                        th = hws.tile([128, 512], BF16, tag="th")
                        ch = hws.tile([128, 512], BF16, tag="ch")
                        nc.scalar.activation(
                            th, pt, AF.Relu, bias=tbsb[:, 1, m:m + 1],
                            scale=1.0 / WSH)
                        nc.scalar.activation(
                            ch, pc, AF.Sigmoid, bias=cbsb[:, 1, m:m + 1],
                            scale=1.0 / WSH)
                        dh = hws.tile([128, 512], BF16, tag="dh")
                        nc.vector.tensor_tensor(
                            dh, th, x1q8[:, m, nsl], op=OP.subtract)
                        mh = hws.tile([128, 512], BF16, tag="mh")
                        nc.vector.tensor_tensor(
                            mh, ch, dh, op=OP.mult)
                        nc.gpsimd.tensor_tensor(
                            x2q8[:, m, nsl], x1q8[:, m, nsl], mh,
                            op=OP.add)
                    if t % 2 == 1:
                        prep_slab(t // 2)

            # ============= Phase D: pairwise softmax attention =============
            fWv = ffW.rearrange("(k p) m -> k p m", p=128)
            rWv = frW.rearrange("(k p) m -> k p m", p=128)
            # x-half (k 0..3) -> bf16 *256 ; att-half (k 4..7) -> fp8 *32
            fuse_chunks = ([(fWv, ffWx, ffW8, k) for k in range(2 * KU)] +
                           [(rWv, frWx, frW8, k) for k in range(2 * KU)])
            with tc.tile_pool(name="pdn", bufs=2, space="PSUM") as pdn, \
                 tc.tile_pool(name="pds", bufs=2, space="PSUM") as pds, \
                 tc.tile_pool(name="pdr", bufs=1, space="PSUM") as pdr, \
                 tc.tile_pool(name="pbc", bufs=1, space="PSUM") as pbc, \
                 tc.tile_pool(name="stgf", bufs=4) as stgf, \
                 tc.tile_pool(name="dsb", bufs=4) as dsb, \
                 tc.tile_pool(name="ehp", bufs=6) as ehp:
                ones2v = ones2c8.rearrange("p (two s) -> p two s", two=2)
                for b in range(BPC):
                    for h in range(IH):
                        # drip-feed fuse-gate weight loads (DMA idle here)
                        unit = b * IH + h
                        for ci in range(unit * 4, unit * 4 + 4):
                            wv_, wbf_, w8_, k_ = fuse_chunks[ci]
                            wsf = stgf.tile([128, U], F32, tag="wsf",
                                            name=f"wsf_{ci}")
                            nc.sync.dma_start(wsf, wv_[k_])
                            if k_ < KU:
                                if ci % 2 == 0:
                                    nc.vector.tensor_scalar_mul(
                                        wbf_[:, k_, :], wsf, WSF)
                                else:
                                    nc.scalar.mul(wbf_[:, k_, :], wsf, WSF)
                            else:
                                if ci % 2 == 0:
                                    nc.vector.tensor_scalar_mul(
                                        w8_[:, k_ - KU, :], wsf, WSH)
                                else:
                                    nc.scalar.mul(w8_[:, k_ - KU, :], wsf,
                                                  WSH)
                        isl = slice(b * L + h * 512, b * L + (h + 1) * 512)
                        # keep-warm burst across the unit boundary
                        wtd = pbc.tile([128, 512], F32, tag="pb",
                                       name=f"wd_{b}_{h}")
                        for i in range(4):
                            nc.tensor.matmul(wtd[:, 0:128], identb, identb,
                                             start=True, stop=True)
                        thbc = dsb.tile([128, 512], BF16, tag="thbc")
                        pb1 = pbc.tile([128, 512], F32, tag="pb",
                                       name=f"pb1_{b}_{h}")
                        nc.tensor.matmul(pb1, ones_row, thr[:, isl],
                                         start=True, stop=True)
                        nc.scalar.copy(thbc, pb1)
                        # ---- stage 1: scores -> exp -> fp8 eh pair tiles
                        ehs = []
                        for p in range(JT // 2):      # j-tile pairs
                            jg = b * JT + 2 * p
                            ps = pds.tile([128, 1024], F32, tag="ps",
                                          name=f"ps_{b}_{h}_{p}")
                            ehb = ehp.tile([128, 1024], F8, tag="ehb",
                                           name=f"ehb_{b}_{h}_{p}")
                            ehbf = dsb.tile([128, 1024], BF16, tag="ehbf")
                            for half in range(2):
                                jsl = slice((jg + half) * 128,
                                            (jg + half + 1) * 128)
                                hsl = slice(half * 512, (half + 1) * 512)
                                for kk in range(KU // 2):
                                    nc.tensor.matmul(
                                        ps[:, hsl], pair(w3x8, kk, jsl),
                                        pair(x2q8, kk, isl), perf_mode=DR,
                                        start=(kk == 0), stop=(kk == 1))
                                nc.scalar.activation(
                                    ehbf[:, hsl], ps[:, hsl], AF.Exp,
                                    bias=s2f[:, jg + half:jg + half + 1],
                                    scale=1.0 / WSA)
                                nc.vector.tensor_tensor(
                                    ehb[:, hsl], ehbf[:, hsl], thbc,
                                    op=OP.max)
                            ehs.append(
                                ehb.rearrange("p (two n) -> p two n", two=2))
                        # ---- stage 2: denominator first, then du-major
                        # numerator accumulation
                        pr = pdr.tile([1, 512], F32, tag="pr")
                        for p in range(JT // 2):
                            nc.tensor.matmul(
                                pr, ones2v[:, :, 0:1], ehs[p], perf_mode=DR,
                                start=(p == 0), stop=(p == JT // 2 - 1))
                        rec = dsb.tile([1, 512], F32, tag="rec")
                        nc.vector.reciprocal_approx_fast(rec, pr)
                        rech = dsb.tile([1, 512], BF16, tag="rech")
                        nc.scalar.mul(rech, rec, ATS)
                        rbc = dsb.tile([128, 512], BF16, tag="rbc")
                        pb2 = pbc.tile([128, 512], F32, tag="pb",
                                       name=f"pb2_{b}_{h}")
                        nc.tensor.matmul(pb2, ones_row, rech,
                                         start=True, stop=True)
                        nc.scalar.copy(rbc, pb2)
                        for du in range(KU):
                            pn = pdn.tile([128, 512], F32, tag="pn",
                                          name=f"pn_{b}_{h}_{du}")
                            for p in range(JT // 2):
                                jg = b * JT + 2 * p
                                nc.tensor.matmul(
                                    pn,
                                    xO8[:, jg:jg + 2,
                                        du * 128:(du + 1) * 128],
                                    ehs[p], perf_mode=DR,
                                    start=(p == 0), stop=(p == JT // 2 - 1))
                            # drain + normalize (*8) in one pass
                            nc.vector.tensor_tensor(
                                attT8[:, du, isl], pn, rbc, op=OP.mult)

            # ============= Phase E: fuse gates + output ====================
            with tc.tile_pool(name="pep", bufs=2, space="PSUM") as pep, \
                 tc.tile_pool(name="peb", bufs=1, space="PSUM") as peb, \
                 tc.tile_pool(name="esb", bufs=3) as esb:
                # broadcast fuse biases (*256) to [128, 512] once
                fbb = esb.tile([128, U], BF16, tag="fbb")
                rbb = esb.tile([128, U], BF16, tag="rbb")
                pfb = peb.tile([128, 512], F32, tag="pfb", name="pfb_f")
                nc.tensor.matmul(pfb, ones_row, ffb_h, start=True, stop=True)
                nc.vector.tensor_copy(fbb, pfb)
                prb = peb.tile([128, 512], F32, tag="pfb", name="pfb_r")
                nc.tensor.matmul(prb, ones_row, frb_h, start=True, stop=True)
                nc.vector.tensor_copy(rbb, prb)
                for mt in range(NT):
                    msl = slice(mt * 128, (mt + 1) * 128)
                    pz = pep.tile([128, 512], F32, tag="pz")
                    pr2 = pep.tile([128, 512], F32, tag="pr2")
                    for k in range(KU):          # x-half, bf16
                        nc.tensor.matmul(pz, xTh[:, k, msl], ffWx[:, k, :],
                                         start=(k == 0), stop=False)
                        nc.tensor.matmul(pr2, xTh[:, k, msl], frWx[:, k, :],
                                         start=(k == 0), stop=False)
                    for kk in range(KU // 2):    # att-half, fp8 DR
                        nc.tensor.matmul(pz, pair(attT8, kk, msl),
                                         pair(ffW8, kk), perf_mode=DR,
                                         start=False, stop=(kk == 1))
                        nc.tensor.matmul(pr2, pair(attT8, kk, msl),
                                         pair(frW8, kk), perf_mode=DR,
                                         start=False, stop=(kk == 1))
                    # bias add on vector (frees psum early), sigmoid on
                    # scalar from SBUF
                    pzs = esb.tile([128, U], BF16, tag="pzs")
                    prs = esb.tile([128, U], BF16, tag="prs")
                    nc.vector.tensor_tensor(pzs, pz, fbb, op=OP.add)
                    nc.vector.tensor_tensor(prs, pr2, rbb, op=OP.add)
                    zh = esb.tile([128, U], BF16, tag="zh")
                    rh = esb.tile([128, U], BF16, tag="rh")
                    q = esb.tile([128, U], F32, tag="q")
                    p2 = esb.tile([128, U], F32, tag="p2")
                    ot = esb.tile([128, U], F32, tag="ot")
                    x0t = x0row[:, mt, :]
                    if mt == NT - 1:
                        # shorten the kernel tail: split across engines
                        hU = U // 2
                        nc.scalar.activation(zh, pzs, AF.Sigmoid,
                                             scale=1.0 / WSF)
                        nc.scalar.square(q, zh)
                        nc.scalar.activation(rh, prs, AF.Sigmoid,
                                             scale=1.0 / WSF)
                        nc.vector.tensor_tensor(p2[:, :hU], rh[:, :hU],
                                                x0t[:, :hU], op=OP.mult)
                        nc.gpsimd.tensor_tensor(p2[:, hU:], rh[:, hU:],
                                                x0t[:, hU:], op=OP.mult)
                        nc.vector.tensor_tensor(ot[:, :hU], q[:, :hU],
                                                p2[:, :hU], op=OP.add)
                        nc.gpsimd.tensor_tensor(ot[:, hU:], q[:, hU:],
                                                p2[:, hU:], op=OP.add)
                    else:
                        nc.scalar.activation(zh, pzs, AF.Sigmoid,
                                             scale=1.0 / WSF)
                        nc.scalar.activation(rh, prs, AF.Sigmoid,
                                             scale=1.0 / WSF)
                        nc.scalar.square(q, zh)
                        nc.vector.tensor_tensor(p2, rh, x0t, op=OP.mult)
                        nc.vector.tensor_tensor(ot, q, p2, op=OP.add)
                    nc.sync.dma_start(outv[mt], ot)

    nc.compile()
    return nc


_NC_CACHE = None


def _get_nc():
    global _NC_CACHE
    if _NC_CACHE is None:
        _NC_CACHE = build_nc()
    return _NC_CACHE


def kernel(**inputs) -> np.ndarray:
    from concourse.bass_utils import run_bass_kernel_spmd

    nc = _get_nc()
    full = {k: np.ascontiguousarray(np.asarray(v, dtype=np.float32))
            for k, v in inputs.items()}
    in_maps = []
    for c in range(NCORES):
        m = dict(full)
        m["inputs"] = np.ascontiguousarray(
            full["inputs"][c * BPC:(c + 1) * BPC])
        in_maps.append(m)
    res = run_bass_kernel_spmd(nc, in_maps, core_ids=list(range(NCORES)))
    return np.concatenate([res.results[c]["out"] for c in range(NCORES)],
                          axis=0)


# revision 48
# speedup vs baseline: 1.0768x; 1.0380x over previous
"""Trainium2 Bass kernel for nn_Encoding_layer (highway stack + pairwise MLP
attention + fuse gates).

Sharding: data-parallel over batch B=16 across 8 NeuronCores (2 batches per
core); all dense weights replicated. No collectives.

fp8-e4m3 DoubleRow matmuls for the compute-heavy GEMMs, with the
schedule restructured to keep the PE HAM clock-gate warm
(HW: 219.1us vs 262.7us baseline; rel err 3.9e-3 vs 2e-2 budget):
  - DoubleRow contracts 256 rows/pass (2 fp8 weights per PE cell); operand
    pairs are adjacent k-tiles in the free dim of the [128, KU, N] tilings.
  - Quantization (numpy-validated, rel err ~3e-3 vs 2e-2 budget):
      highway (x fp8, W fp8*32)   scores s3 (w3x fp8*64 x x2 fp8)
      att numerator (xO fp8 x eh fp8)   att stored fp8*8
      fuse gates: x-half bf16 (W bf16*256) + att-half fp8 DR (att*8 x W*32)
    All scales undone via scalar.activation(func, scale=2^-k).
  - Highway layer 0 is merged into the load phase: each slab's matmuls are
    emitted as soon as its token group + weights land, so l0 computes
    during the remaining input DMA stream.
  - Attention prep (row-major transposes, s1/s2/thr) is emitted per-slab
    inside highway layer 1, filling PE slack in the elementwise-bound
    highway stretch; keep-warm matmul bursts bridge the known idle
    valleys so HAM stays at K=8/8.
  - Phase D per (b,h) unit is two-staged: (1) all four j-tile-pair score
    blocks -> exp -> fp8 eh pair tiles (SBUF), (2) denominator then
    du-major numerator accumulation, so wide score psum is double-buffered
    within the 8-bank budget.
  - eh pair tiles [128,2,512] fp8 are exactly the DoubleRow moving operand
    of the numerator.  relu-as-clamp: M^T = max(exp(s3+s2), exp(-(s1+ab)))
    (the per-column factor exp(s1+ab) cancels in the softmax).
"""

import numpy as np

B, L, U, H = 16, 1024, 512, 2
NCORES = 8
BPC = B // NCORES          # batches per core
N = BPC * L                # token columns per core
KU = U // 128              # 4  u-tiles
NT = N // 128              # 16 row-tiles per core
NS = N // 512              # 4  512-wide column slices per core
JT = L // 128              # 8  j-tiles per batch
IH = L // 512              # 2  i-halves per batch

WSH = 32.0                 # highway weight prescale (2^5)
WSA = 64.0                 # aW prescale (2^6)
WSF = 256.0                # fuse-gate effective prescale (2^8)
ATS = 8.0                  # att fp8 prescale (2^3)


def build_nc():
    import concourse.bacc as bacc
    import concourse.tile as tile
    from concourse import mybir
    from concourse.masks import make_identity

    F32 = mybir.dt.float32
    BF16 = mybir.dt.bfloat16
    F8 = mybir.dt.float8e4
    AF = mybir.ActivationFunctionType
    OP = mybir.AluOpType
    DR = mybir.MatmulPerfMode.DoubleRow

    nc = bacc.Bacc("TRN2", target_bir_lowering=False, debug=False,
                   num_devices=NCORES)

    x_in = nc.dram_tensor("inputs", [BPC, L, U], F32, kind="ExternalInput").ap()
    tW = nc.dram_tensor("tW", [H, U, U], F32, kind="ExternalInput").ap()
    tb = nc.dram_tensor("tb", [H, U], F32, kind="ExternalInput").ap()
    cW = nc.dram_tensor("cW", [H, U, U], F32, kind="ExternalInput").ap()
    cb = nc.dram_tensor("cb", [H, U], F32, kind="ExternalInput").ap()
    aW = nc.dram_tensor("aW", [3 * U], F32, kind="ExternalInput").ap()
    ab = nc.dram_tensor("ab", [1], F32, kind="ExternalInput").ap()
    frW = nc.dram_tensor("frW", [2 * U, U], F32, kind="ExternalInput").ap()
    frb = nc.dram_tensor("frb", [U], F32, kind="ExternalInput").ap()
    ffW = nc.dram_tensor("ffW", [2 * U, U], F32, kind="ExternalInput").ap()
    ffb = nc.dram_tensor("ffb", [U], F32, kind="ExternalInput").ap()
    out = nc.dram_tensor("out", [BPC, L, U], F32, kind="ExternalOutput").ap()

    xv = x_in.flatten_outer_dims().rearrange("(t p) u -> t p u", p=128)
    outv = out.flatten_outer_dims().rearrange("(t p) u -> t p u", p=128)

    def pair(t, k2, sl=None):
        """[128, 2, *] DoubleRow view of adjacent k-tiles k2*2, k2*2+1."""
        return t[:, 2 * k2:2 * k2 + 2, sl] if sl is not None \
            else t[:, 2 * k2:2 * k2 + 2, :]

    with tile.TileContext(nc) as tc:
        with tc.tile_pool(name="pers", bufs=1) as pers:
            # ---- persistent SBUF tensors ----
            x0row = pers.tile([128, NT, U], F32, tag="x0row")  # inputs row-maj
            xTh = pers.tile([128, KU, N], BF16, tag="xTh")     # inputs^T bf16
            x0q8 = pers.tile([128, KU, N], F8, tag="x0q8")     # inputs^T fp8
            x1q8 = pers.tile([128, KU, N], F8, tag="x1q8")
            x2q8 = pers.tile([128, KU, N], F8, tag="x2q8")
            w3x8 = pers.tile([128, KU, N], F8, tag="w3x8")     # (w3*64)*x2^T
            attT8 = pers.tile([128, KU, N], F8, tag="attT8")   # att^T * 8
            xO8 = pers.tile([128, NT, U], F8, tag="xO8")       # row-major x2
            tWh8 = pers.tile([128, H, KU, U], F8, tag="tWh8")  # *32
            cWh8 = pers.tile([128, H, KU, U], F8, tag="cWh8")  # *32
            ffWx = pers.tile([128, KU, U], BF16, tag="ffWx")   # x-half *256
            frWx = pers.tile([128, KU, U], BF16, tag="frWx")
            ffW8 = pers.tile([128, KU, U], F8, tag="ffW8")     # att-half *32
            frW8 = pers.tile([128, KU, U], F8, tag="frW8")
            ffx8 = pers.tile([128, 2, U], F8, tag="ffx8")      # x k0-1 *256
            frx8 = pers.tile([128, 2, U], F8, tag="frx8")
            tbsb = pers.tile([128, H, KU], F32, tag="tbsb")
            cbsb = pers.tile([128, H, KU], F32, tag="cbsb")
            awsb = pers.tile([128, 12], F32, tag="awsb")       # w1|w2|w3 cols
            w1h8 = pers.tile([128, KU, 16], F8, tag="w1h8")    # *64, col 0
            w2h8 = pers.tile([128, KU, 16], F8, tag="w2h8")    # *64, col 0
            aw3s = pers.tile([128, KU], F32, tag="aw3s")       # w3 * 64 f32
            ab_sb = pers.tile([1, 1], F32, tag="ab_sb")
            nab_sb = pers.tile([1, 1], F32, tag="nab_sb")
            ffb_h = pers.tile([1, U], BF16, tag="ffb_h")       # *256
            frb_h = pers.tile([1, U], BF16, tag="frb_h")       # *256
            thr = pers.tile([1, N], BF16, tag="thr")   # exp(-(s1+ab))
            s2f = pers.tile([128, NT], F32, tag="s2f")
            ones_row = pers.tile([1, 128], BF16, tag="ones_row")
            ones2c8 = pers.tile([128, 32], F8, tag="ones2c8")  # DR ones pairs
            identb = pers.tile([128, 128], BF16, tag="identb")
            ident8 = pers.tile([128, 128], F8, tag="ident8")
            identf = pers.tile([128, 128], F32, tag="identf")

            nc.vector.memset(ones_row, 1.0)
            nc.vector.memset(ones2c8, 1.0)
            make_identity(nc, identb)
            make_identity(nc, ident8)
            make_identity(nc, identf)

            # ===== Phase A + highway layer 0, merged ======================
            # l0 slabs are emitted as soon as their token group and weights
            # land, so l0's matmuls run during the tg2/tg3 DMA stream
            # instead of as a separate dense block afterwards.
            with tc.tile_pool(name="stg", bufs=2) as stg, \
                 tc.tile_pool(name="stgw", bufs=2) as stgw, \
                 tc.tile_pool(name="hb0", bufs=2, space="PSUM") as hb0, \
                 tc.tile_pool(name="hs0", bufs=3) as hs0, \
                 tc.tile_pool(name="ptA", bufs=1, space="PSUM") as ptA:

                def keep_warm(n, who):
                    warmp = ptA.tile([128, 512], F32, tag="ptk0",
                                     name=f"warm_{who}")
                    for i in range(n):
                        nc.tensor.matmul(warmp[:, 0:128], identb, identb,
                                         start=True, stop=True)

                def l0_slab(t):
                    nsl = slice(t * 512, (t + 1) * 512)
                    for m in range(KU):
                        msl = slice(m * 128, (m + 1) * 128)
                        pt = hb0.tile([128, 512], F32, tag="pt")
                        pc = hb0.tile([128, 512], F32, tag="pc")
                        for kk in range(KU // 2):
                            nc.tensor.matmul(
                                pt, pair(tWh8[:, 0], kk, msl),
                                pair(x0q8, kk, nsl), perf_mode=DR,
                                start=(kk == 0), stop=(kk == 1))
                        for kk in range(KU // 2):
                            nc.tensor.matmul(
                                pc, pair(cWh8[:, 0], kk, msl),
                                pair(x0q8, kk, nsl), perf_mode=DR,
                                start=(kk == 0), stop=(kk == 1))
                        th = hs0.tile([128, 512], BF16, tag="th")
                        ch = hs0.tile([128, 512], BF16, tag="ch")
                        nc.scalar.activation(
                            th, pt, AF.Relu, bias=tbsb[:, 0, m:m + 1],
                            scale=1.0 / WSH)
                        nc.scalar.activation(
                            ch, pc, AF.Sigmoid, bias=cbsb[:, 0, m:m + 1],
                            scale=1.0 / WSH)
                        dh = hs0.tile([128, 512], BF16, tag="dh")
                        nc.vector.tensor_tensor(
                            dh, th, x0q8[:, m, nsl], op=OP.subtract)
                        mh = hs0.tile([128, 512], BF16, tag="mh")
                        nc.vector.tensor_tensor(mh, ch, dh, op=OP.mult)
                        nc.gpsimd.tensor_tensor(
                            x1q8[:, m, nsl], x0q8[:, m, nsl], mh,
                            op=OP.add)

                # highway-weight loads interleaved after tg0/tg1 so layer-0
                # can start as soon as the first column group lands; one
                # 1MB DMA + one wide cast per (layer, gate)
                def emit_weights(l, wi):
                    wsrc, wdst = ((tW, tWh8), (cW, cWh8))[wi]
                    wv = wsrc[l].rearrange("(k p) m -> p k m", p=128)
                    ws = stgw.tile([128, KU, U], F32, tag="ws",
                                   name=f"ws_{l}_{wi}")
                    nc.sync.dma_start(ws, wv)
                    if wi == 0:
                        nc.vector.tensor_scalar_mul(wdst[:, l], ws, WSH)
                    else:
                        nc.scalar.mul(wdst[:, l], ws, WSH)

                # warm the PE HAM clock-gate during the initial DMA wait
                keep_warm(32, "init")
                for tg in range(NS):
                    # one 1MB DMA per 512-token group, straight into the
                    # persistent row-major copy (reused by phase E)
                    nc.sync.dma_start(
                        x0row[:, 4 * tg:4 * tg + 4, :],
                        x_in.flatten_outer_dims().rearrange(
                            "(t p) u -> p t u", p=128)[:, 4 * tg:4 * tg + 4])
                    ptk = [ptA.tile([128, 512], F32, tag=f"ptk{k}",
                                    name=f"ptk_{tg}_{k}")
                           for k in range(KU)]
                    for tt in range(4):
                        t = tg * 4 + tt
                        for k in range(KU):
                            nc.tensor.transpose(
                                ptk[k][:, tt * 128:(tt + 1) * 128],
                                x0row[:, t, k * 128:(k + 1) * 128], identf)
                    for k in range(KU):
                        sl = slice(tg * 512, (tg + 1) * 512)
                        if k % 2 == 0:
                            nc.vector.tensor_copy(xTh[:, k, sl], ptk[k])
                            nc.scalar.copy(x0q8[:, k, sl], ptk[k])
                        else:
                            nc.scalar.copy(xTh[:, k, sl], ptk[k])
                            nc.vector.tensor_copy(x0q8[:, k, sl], ptk[k])
                    keep_warm(6, f"tg{tg}")
                    if tg == 0:
                        nc.sync.dma_start(
                            tbsb, tb.rearrange("l (m p) -> p l m", p=128))
                        nc.sync.dma_start(
                            cbsb, cb.rearrange("l (m p) -> p l m", p=128))
                        nc.sync.dma_start(
                            awsb, aW.rearrange("(w m p) -> p (w m)",
                                               p=128, w=3))
                        for k in range(KU):
                            nc.vector.tensor_scalar_mul(
                                w1h8[:, k, 0:1], awsb[:, k:k + 1], WSA)
                            nc.vector.tensor_scalar_mul(
                                w2h8[:, k, 0:1], awsb[:, KU + k:KU + k + 1],
                                WSA)
                            nc.scalar.mul(aw3s[:, k:k + 1],
                                          awsb[:, 8 + k:9 + k], WSA)
                        nc.sync.dma_start(ab_sb, ab[None, :])
                        nc.scalar.mul(nab_sb, ab_sb, -1.0)
                        fb = stg.tile([1, U], F32, tag="fb")
                        nc.sync.dma_start(fb, ffb[None, :])
                        nc.vector.tensor_scalar_mul(ffb_h, fb, WSF)
                        fb2 = stg.tile([1, U], F32, tag="fb")
                        nc.sync.dma_start(fb2, frb[None, :])
                        nc.vector.tensor_scalar_mul(frb_h, fb2, WSF)
                        emit_weights(0, 0)
                    elif tg == 1:
                        emit_weights(0, 1)
                        l0_slab(0)
                    elif tg == H:
                        emit_weights(1, 0)
                        l0_slab(1)
                    else:
                        emit_weights(1, 1)
                        l0_slab(2)
                        l0_slab(3)

            # ===== Phase B layer 0: wide [128,1024] 2-bank psum tiles ======
            with tc.tile_pool(name="hw0", bufs=2, space="PSUM") as hw0, \
                 tc.tile_pool(name="hs0", bufs=3) as hs0:
                for tp in range(NS // 2):              # 1024-token slabs
                    wsl = slice(tp * 1024, (tp + 1) * 1024)
                    for m in range(KU):
                        msl = slice(m * 128, (m + 1) * 128)
                        pt = hw0.tile([128, 1024], F32, tag="pt")
                        pc = hw0.tile([128, 1024], F32, tag="pc")
                        for h2 in range(2):
                            nsl = slice(tp * 1024 + h2 * 512,
                                        tp * 1024 + (h2 + 1) * 512)
                            psl = slice(h2 * 512, (h2 + 1) * 512)
                            for kk in range(KU // 2):
                                nc.tensor.matmul(
                                    pt[:, psl], pair(tWh8[:, 0], kk, msl),
                                    pair(x0q8, kk, nsl), perf_mode=DR,
                                    start=(kk == 0), stop=(kk == 1))
                            for kk in range(KU // 2):
                                nc.tensor.matmul(
                                    pc[:, psl], pair(cWh8[:, 0], kk, msl),
                                    pair(x0q8, kk, nsl), perf_mode=DR,
                                    start=(kk == 0), stop=(kk == 1))
                        th = hs0.tile([128, 1024], BF16, tag="th")
                        ch = hs0.tile([128, 1024], BF16, tag="ch")
                        nc.scalar.activation(
                            th, pt, AF.Relu, bias=tbsb[:, 0, m:m + 1],
                            scale=1.0 / WSH)
                        nc.scalar.activation(
                            ch, pc, AF.Sigmoid, bias=cbsb[:, 0, m:m + 1],
                            scale=1.0 / WSH)
                        dh = hs0.tile([128, 1024], BF16, tag="dh")
                        nc.vector.tensor_tensor(
                            dh, th, x0q8[:, m, wsl], op=OP.subtract)
                        mh = hs0.tile([128, 1024], BF16, tag="mh")
                        nc.vector.tensor_tensor(mh, ch, dh, op=OP.mult)
                        nc.gpsimd.tensor_tensor(
                            x1q8[:, m, wsl], x0q8[:, m, wsl], mh, op=OP.add)

            # ===== Phase B layer 1 + C: highway + per-slab attention prep ==
            # During layer 1, each finished 1024-token slab immediately gets
            # its row-major transposes, w3x, s1/thr and s2 emitted, filling
            # PE slack in the elementwise-bound highway stretch.
            with tc.tile_pool(name="hwp", bufs=2, space="PSUM") as hwp, \
                 tc.tile_pool(name="pcp", bufs=2, space="PSUM") as pcp, \
                 tc.tile_pool(name="pcp1", bufs=1, space="PSUM") as pcp1, \
                 tc.tile_pool(name="hws", bufs=3) as hws:
                s2p = pcp1.tile([128, NT], F32, tag="s2p")

                def prep_slab(tp):
                    """attention prep for tokens [tp*1024, (tp+1)*1024)."""
                    for k in range(KU):
                        wsl = slice(tp * 1024, (tp + 1) * 1024)
                        nc.vector.tensor_scalar_mul(
                            w3x8[:, k, wsl], x2q8[:, k, wsl],
                            aw3s[:, k:k + 1])
                    for jt in range(8 * tp, 8 * tp + 8):
                        # full-bank staging tile so rotating bufs land in
                        # different banks (PE-write vs DVE-read collision)
                        ptr = pcp.tile([128, 2048], F8, tag="ptr")
                        ptv = ptr[:, 0:1024].rearrange(
                            "p (n two) -> p n two", two=2)
                        for k in range(KU):
                            nc.tensor.transpose(
                                ptv[:, k * 128:(k + 1) * 128, 0:1],
                                x2q8[:, k, jt * 128:(jt + 1) * 128], ident8)
                        if jt % 2 == 0:
                            nc.vector.tensor_copy(xO8[:, jt, :],
                                                  ptv[:, :, 0:1])
                        else:
                            nc.scalar.copy(xO8[:, jt, :], ptv[:, :, 0:1])
                    for t in (2 * tp, 2 * tp + 1):
                        ps1 = pcp1.tile([1, 512], F32, tag="ps1")
                        for kk in range(KU // 2):
                            nc.tensor.matmul(
                                ps1, pair(w1h8, kk, slice(0, 1)),
                                pair(x2q8, kk,
                                     slice(t * 512, (t + 1) * 512)),
                                perf_mode=DR,
                                start=(kk == 0), stop=(kk == 1))
                        nc.scalar.activation(
                            thr[:, t * 512:(t + 1) * 512], ps1, AF.Exp,
                            bias=nab_sb, scale=-1.0 / WSA)
                    for jt in range(8 * tp, 8 * tp + 8):
                        jsl = slice(jt * 128, (jt + 1) * 128)
                        for kk in range(KU // 2):
                            nc.tensor.matmul(
                                s2p[:, jt:jt + 1],
                                pair(x2q8, kk, jsl),
                                pair(w2h8, kk, slice(0, 1)),
                                perf_mode=DR,
                                start=(kk == 0), stop=(kk == 1))
                    nc.scalar.mul(s2f[:, 8 * tp:8 * tp + 8],
                                  s2p[:, 8 * tp:8 * tp + 8], 1.0 / WSA)

                for t in range(NS):                    # 512-token slabs
                    nsl = slice(t * 512, (t + 1) * 512)
                    for m in range(KU):
                        if t == 0:
                            # keep the PE HAM clock-gate warm through the
                            # l0-tail / l1-ramp valley (prep pool is idle)
                            wt = pcp.tile([128, 2048], F8, tag="ptr",
                                          name=f"wl1_{m}").bitcast(F32)
                            for i in range(4):
                                nc.tensor.matmul(wt[:, 0:128], identb,
                                                 identb, start=True,
                                                 stop=True)
                        msl = slice(m * 128, (m + 1) * 128)
                        pt = hwp.tile([128, 512], F32, tag="pt")
                        pc = hwp.tile([128, 512], F32, tag="pc")
                        for kk in range(KU // 2):
                            nc.tensor.matmul(
                                pt, pair(tWh8[:, 1], kk, msl),
                                pair(x1q8, kk, nsl), perf_mode=DR,
                                start=(kk == 0), stop=(kk == 1))
                        for kk in range(KU // 2):
                            nc.tensor.matmul(
                                pc, pair(cWh8[:, 1], kk, msl),
                                pair(x1q8, kk, nsl), perf_mode=DR,
                                start=(kk == 0), stop=(kk == 1))
                        th = hws.tile([128, 512], BF16, tag="th")
                        ch = hws.tile([128, 512], BF16, tag="ch")
                        nc.scalar.activation(
                            th, pt, AF.Relu, bias=tbsb[:, 1, m:m + 1],
                            scale=1.0 / WSH)
                        nc.scalar.activation(
                            ch, pc, AF.Sigmoid, bias=cbsb[:, 1, m:m + 1],
                            scale=1.0 / WSH)
                        dh = hws.tile([128, 512], BF16, tag="dh")
                        nc.vector.tensor_tensor(
                            dh, th, x1q8[:, m, nsl], op=OP.subtract)
                        mh = hws.tile([128, 512], BF16, tag="mh")
                        nc.vector.tensor_tensor(
                            mh, ch, dh, op=OP.mult)
                        nc.gpsimd.tensor_tensor(
                            x2q8[:, m, nsl], x1q8[:, m, nsl], mh,
                            op=OP.add)
                    if t % 2 == 1:
                        prep_slab(t // 2)

            # ============= Phase D: pairwise softmax attention =============
            fWv = ffW.rearrange("(k p) m -> k p m", p=128)
            rWv = frW.rearrange("(k p) m -> k p m", p=128)
            # x-half (k 0..3) -> bf16 *256 ; att-half (k 4..7) -> fp8 *32
            fuse_chunks = ([(fWv, ffWx, ffW8, k) for k in range(2 * KU)] +
                           [(rWv, frWx, frW8, k) for k in range(2 * KU)])
            with tc.tile_pool(name="pdn", bufs=2, space="PSUM") as pdn, \
                 tc.tile_pool(name="pds", bufs=2, space="PSUM") as pds, \
                 tc.tile_pool(name="pdr", bufs=1, space="PSUM") as pdr, \
                 tc.tile_pool(name="pbc", bufs=1, space="PSUM") as pbc, \
                 tc.tile_pool(name="stgf", bufs=4) as stgf, \
                 tc.tile_pool(name="dsb", bufs=4) as dsb, \
                 tc.tile_pool(name="ehp", bufs=6) as ehp:
                ones2v = ones2c8.rearrange("p (two s) -> p two s", two=2)
                for b in range(BPC):
                    for h in range(IH):
                        # drip-feed fuse-gate weight loads (DMA idle here)
                        unit = b * IH + h
                        for ci in range(unit * 4, unit * 4 + 4):
                            wv_, wbf_, w8_, k_ = fuse_chunks[ci]
                            wsf = stgf.tile([128, U], F32, tag="wsf",
                                            name=f"wsf_{ci}")
                            nc.sync.dma_start(wsf, wv_[k_])
                            if k_ < KU:
                                if ci % 2 == 0:
                                    nc.vector.tensor_scalar_mul(
                                        wbf_[:, k_, :], wsf, WSF)
                                else:
                                    nc.scalar.mul(wbf_[:, k_, :], wsf, WSF)
                                if k_ < 2:   # fp8 copy for the DR x-half
                                    x8d = ffx8 if ci < 2 * KU else frx8
                                    nc.scalar.copy(x8d[:, k_, :],
                                                   wbf_[:, k_, :])
                            else:
                                if ci % 2 == 0:
                                    nc.vector.tensor_scalar_mul(
                                        w8_[:, k_ - KU, :], wsf, WSH)
                                else:
                                    nc.scalar.mul(w8_[:, k_ - KU, :], wsf,
                                                  WSH)
                        isl = slice(b * L + h * 512, b * L + (h + 1) * 512)
                        # keep-warm burst across the unit boundary
                        wtd = pbc.tile([128, 512], F32, tag="pb",
                                       name=f"wd_{b}_{h}")
                        for i in range(4):
                            nc.tensor.matmul(wtd[:, 0:128], identb, identb,
                                             start=True, stop=True)
                        thbc = dsb.tile([128, 512], BF16, tag="thbc")
                        pb1 = pbc.tile([128, 512], F32, tag="pb",
                                       name=f"pb1_{b}_{h}")
                        nc.tensor.matmul(pb1, ones_row, thr[:, isl],
                                         start=True, stop=True)
                        nc.scalar.copy(thbc, pb1)
                        # ---- stage 1: scores -> exp -> fp8 eh pair tiles
                        ehs = []
                        for p in range(JT // 2):      # j-tile pairs
                            jg = b * JT + 2 * p
                            ps = pds.tile([128, 1024], F32, tag="ps",
                                          name=f"ps_{b}_{h}_{p}")
                            ehb = ehp.tile([128, 1024], F8, tag="ehb",
                                           name=f"ehb_{b}_{h}_{p}")
                            ehbf = dsb.tile([128, 1024], BF16, tag="ehbf")
                            for half in range(2):
                                jsl = slice((jg + half) * 128,
                                            (jg + half + 1) * 128)
                                hsl = slice(half * 512, (half + 1) * 512)
                                for kk in range(KU // 2):
                                    nc.tensor.matmul(
                                        ps[:, hsl], pair(w3x8, kk, jsl),
                                        pair(x2q8, kk, isl), perf_mode=DR,
                                        start=(kk == 0), stop=(kk == 1))
                                nc.scalar.activation(
                                    ehbf[:, hsl], ps[:, hsl], AF.Exp,
                                    bias=s2f[:, jg + half:jg + half + 1],
                                    scale=1.0 / WSA)
                                nc.vector.tensor_tensor(
                                    ehb[:, hsl], ehbf[:, hsl], thbc,
                                    op=OP.max)
                            ehs.append(
                                ehb.rearrange("p (two n) -> p two n", two=2))
                        # ---- stage 2: denominator first, then du-major
                        # numerator accumulation
                        pr = pdr.tile([1, 512], F32, tag="pr")
                        for p in range(JT // 2):
                            nc.tensor.matmul(
                                pr, ones2v[:, :, 0:1], ehs[p], perf_mode=DR,
                                start=(p == 0), stop=(p == JT // 2 - 1))
                        rec = dsb.tile([1, 512], F32, tag="rec")
                        nc.vector.reciprocal_approx_fast(rec, pr)
                        rech = dsb.tile([1, 512], BF16, tag="rech")
                        nc.scalar.mul(rech, rec, ATS)
                        rbc = dsb.tile([128, 512], BF16, tag="rbc")
                        pb2 = pbc.tile([128, 512], F32, tag="pb",
                                       name=f"pb2_{b}_{h}")
                        nc.tensor.matmul(pb2, ones_row, rech,
                                         start=True, stop=True)
                        nc.scalar.copy(rbc, pb2)
                        for du in range(KU):
                            pn = pdn.tile([128, 512], F32, tag="pn",
                                          name=f"pn_{b}_{h}_{du}")
                            for p in range(JT // 2):
                                jg = b * JT + 2 * p
                                nc.tensor.matmul(
                                    pn,
                                    xO8[:, jg:jg + 2,
                                        du * 128:(du + 1) * 128],
                                    ehs[p], perf_mode=DR,
                                    start=(p == 0), stop=(p == JT // 2 - 1))
                            # drain + normalize (*8) in one pass
                            nc.vector.tensor_tensor(
                                attT8[:, du, isl], pn, rbc, op=OP.mult)

            # ============= Phase E: fuse gates + output ====================
            with tc.tile_pool(name="pep", bufs=2, space="PSUM") as pep, \
                 tc.tile_pool(name="peb", bufs=1, space="PSUM") as peb, \
                 tc.tile_pool(name="esb", bufs=3) as esb:
                # broadcast fuse biases (*256) to [128, 512] once
                fbb = esb.tile([128, U], BF16, tag="fbb")
                rbb = esb.tile([128, U], BF16, tag="rbb")
                pfb = peb.tile([128, 512], F32, tag="pfb", name="pfb_f")
                nc.tensor.matmul(pfb, ones_row, ffb_h, start=True, stop=True)
                nc.vector.tensor_copy(fbb, pfb)
                prb = peb.tile([128, 512], F32, tag="pfb", name="pfb_r")
                nc.tensor.matmul(prb, ones_row, frb_h, start=True, stop=True)
                nc.vector.tensor_copy(rbb, prb)
                for mt in range(NT):
                    msl = slice(mt * 128, (mt + 1) * 128)
                    pz = pep.tile([128, 512], F32, tag="pz")
                    pr2 = pep.tile([128, 512], F32, tag="pr2")
                    # x-half: k-tiles 0-1 as one fp8 DR pass, 2-3 bf16
                    nc.tensor.matmul(pz, pair(x0q8, 0, msl), ffx8,
                                     perf_mode=DR, start=True, stop=False)
                    nc.tensor.matmul(pr2, pair(x0q8, 0, msl), frx8,
                                     perf_mode=DR, start=True, stop=False)
                    for k in (2, 3):
                        nc.tensor.matmul(pz, xTh[:, k, msl], ffWx[:, k, :],
                                         start=False, stop=False)
                        nc.tensor.matmul(pr2, xTh[:, k, msl], frWx[:, k, :],
                                         start=False, stop=False)
                    for kk in range(KU // 2):    # att-half, fp8 DR
                        nc.tensor.matmul(pz, pair(attT8, kk, msl),
                                         pair(ffW8, kk), perf_mode=DR,
                                         start=False, stop=(kk == 1))
                        nc.tensor.matmul(pr2, pair(attT8, kk, msl),
                                         pair(frW8, kk), perf_mode=DR,
                                         start=False, stop=(kk == 1))
                    # bias add on vector (frees psum early), sigmoid on
                    # scalar from SBUF
                    pzs = esb.tile([128, U], BF16, tag="pzs")
                    prs = esb.tile([128, U], BF16, tag="prs")
                    nc.vector.tensor_tensor(pzs, pz, fbb, op=OP.add)
                    nc.vector.tensor_tensor(prs, pr2, rbb, op=OP.add)
                    zh = esb.tile([128, U], BF16, tag="zh")
                    rh = esb.tile([128, U], BF16, tag="rh")
                    q = esb.tile([128, U], F32, tag="q")
                    p2 = esb.tile([128, U], F32, tag="p2")
                    ot = esb.tile([128, U], F32, tag="ot")
                    x0t = x0row[:, mt, :]
                    if mt == NT - 1:
                        # shorten the kernel tail: split across engines
                        hU = U // 2
                        nc.scalar.activation(zh, pzs, AF.Sigmoid,
                                             scale=1.0 / WSF)
                        nc.scalar.square(q, zh)
                        nc.scalar.activation(rh, prs, AF.Sigmoid,
                                             scale=1.0 / WSF)
                        nc.vector.tensor_tensor(p2[:, :hU], rh[:, :hU],
                                                x0t[:, :hU], op=OP.mult)
                        nc.gpsimd.tensor_tensor(p2[:, hU:], rh[:, hU:],
                                                x0t[:, hU:], op=OP.mult)
                        nc.vector.tensor_tensor(ot[:, :hU], q[:, :hU],
                                                p2[:, :hU], op=OP.add)
                        nc.gpsimd.tensor_tensor(ot[:, hU:], q[:, hU:],
                                                p2[:, hU:], op=OP.add)
                    else:
                        nc.scalar.activation(zh, pzs, AF.Sigmoid,
                                             scale=1.0 / WSF)
                        nc.scalar.activation(rh, prs, AF.Sigmoid,
                                             scale=1.0 / WSF)
                        nc.scalar.square(q, zh)
                        nc.vector.tensor_tensor(p2, rh, x0t, op=OP.mult)
                        nc.gpsimd.tensor_tensor(ot, q, p2, op=OP.add)
                    nc.sync.dma_start(outv[mt], ot)

    nc.compile()
    return nc


_NC_CACHE = None


def _get_nc():
    global _NC_CACHE
    if _NC_CACHE is None:
        _NC_CACHE = build_nc()
    return _NC_CACHE


def kernel(**inputs) -> np.ndarray:
    from concourse.bass_utils import run_bass_kernel_spmd

    nc = _get_nc()
    full = {k: np.ascontiguousarray(np.asarray(v, dtype=np.float32))
            for k, v in inputs.items()}
    in_maps = []
    for c in range(NCORES):
        m = dict(full)
        m["inputs"] = np.ascontiguousarray(
            full["inputs"][c * BPC:(c + 1) * BPC])
        in_maps.append(m)
    res = run_bass_kernel_spmd(nc, in_maps, core_ids=list(range(NCORES)))
    return np.concatenate([res.results[c]["out"] for c in range(NCORES)],
                          axis=0)


# revision 49
# speedup vs baseline: 1.0820x; 1.0048x over previous
"""Trainium2 Bass kernel for nn_Encoding_layer (highway stack + pairwise MLP
attention + fuse gates).

Sharding: data-parallel over batch B=16 across 8 NeuronCores (2 batches per
core); all dense weights replicated. No collectives.

fp8-e4m3 DoubleRow matmuls for the compute-heavy GEMMs, with the
schedule restructured to keep the PE HAM clock-gate warm
(HW: 219.1us vs 262.7us baseline; rel err 3.9e-3 vs 2e-2 budget):
  - DoubleRow contracts 256 rows/pass (2 fp8 weights per PE cell); operand
    pairs are adjacent k-tiles in the free dim of the [128, KU, N] tilings.
  - Quantization (numpy-validated, rel err ~3e-3 vs 2e-2 budget):
      highway (x fp8, W fp8*32)   scores s3 (w3x fp8*64 x x2 fp8)
      att numerator (xO fp8 x eh fp8)   att stored fp8*8
      fuse gates: x-half bf16 (W bf16*256) + att-half fp8 DR (att*8 x W*32)
    All scales undone via scalar.activation(func, scale=2^-k).
  - Highway layer 0 is merged into the load phase: each slab's matmuls are
    emitted as soon as its token group + weights land, so l0 computes
    during the remaining input DMA stream.
  - Attention prep (row-major transposes, s1/s2/thr) is emitted per-slab
    inside highway layer 1, filling PE slack in the elementwise-bound
    highway stretch; keep-warm matmul bursts bridge the known idle
    valleys so HAM stays at K=8/8.
  - Phase D per (b,h) unit is two-staged: (1) all four j-tile-pair score
    blocks -> exp -> fp8 eh pair tiles (SBUF), (2) denominator then
    du-major numerator accumulation, so wide score psum is double-buffered
    within the 8-bank budget.
  - eh pair tiles [128,2,512] fp8 are exactly the DoubleRow moving operand
    of the numerator.  relu-as-clamp: M^T = max(exp(s3+s2), exp(-(s1+ab)))
    (the per-column factor exp(s1+ab) cancels in the softmax).
"""

import numpy as np

B, L, U, H = 16, 1024, 512, 2
NCORES = 8
BPC = B // NCORES          # batches per core
N = BPC * L                # token columns per core
KU = U // 128              # 4  u-tiles
NT = N // 128              # 16 row-tiles per core
NS = N // 512              # 4  512-wide column slices per core
JT = L // 128              # 8  j-tiles per batch
IH = L // 512              # 2  i-halves per batch

WSH = 32.0                 # highway weight prescale (2^5)
WSA = 64.0                 # aW prescale (2^6)
WSF = 256.0                # fuse-gate effective prescale (2^8)
ATS = 8.0                  # att fp8 prescale (2^3)


def build_nc():
    import concourse.bacc as bacc
    import concourse.tile as tile
    from concourse import mybir
    from concourse.masks import make_identity

    F32 = mybir.dt.float32
    BF16 = mybir.dt.bfloat16
    F8 = mybir.dt.float8e4
    AF = mybir.ActivationFunctionType
    OP = mybir.AluOpType
    DR = mybir.MatmulPerfMode.DoubleRow

    nc = bacc.Bacc("TRN2", target_bir_lowering=False, debug=False,
                   num_devices=NCORES)

    x_in = nc.dram_tensor("inputs", [BPC, L, U], F32, kind="ExternalInput").ap()
    tW = nc.dram_tensor("tW", [H, U, U], F32, kind="ExternalInput").ap()
    tb = nc.dram_tensor("tb", [H, U], F32, kind="ExternalInput").ap()
    cW = nc.dram_tensor("cW", [H, U, U], F32, kind="ExternalInput").ap()
    cb = nc.dram_tensor("cb", [H, U], F32, kind="ExternalInput").ap()
    aW = nc.dram_tensor("aW", [3 * U], F32, kind="ExternalInput").ap()
    ab = nc.dram_tensor("ab", [1], F32, kind="ExternalInput").ap()
    frW = nc.dram_tensor("frW", [2 * U, U], F32, kind="ExternalInput").ap()
    frb = nc.dram_tensor("frb", [U], F32, kind="ExternalInput").ap()
    ffW = nc.dram_tensor("ffW", [2 * U, U], F32, kind="ExternalInput").ap()
    ffb = nc.dram_tensor("ffb", [U], F32, kind="ExternalInput").ap()
    out = nc.dram_tensor("out", [BPC, L, U], F32, kind="ExternalOutput").ap()

    xv = x_in.flatten_outer_dims().rearrange("(t p) u -> t p u", p=128)
    outv = out.flatten_outer_dims().rearrange("(t p) u -> t p u", p=128)

    def pair(t, k2, sl=None):
        """[128, 2, *] DoubleRow view of adjacent k-tiles k2*2, k2*2+1."""
        return t[:, 2 * k2:2 * k2 + 2, sl] if sl is not None \
            else t[:, 2 * k2:2 * k2 + 2, :]

    with tile.TileContext(nc) as tc:
        with tc.tile_pool(name="pers", bufs=1) as pers:
            # ---- persistent SBUF tensors ----
            x0row = pers.tile([128, NT, U], F32, tag="x0row")  # inputs row-maj
            xTh = pers.tile([128, KU, N], BF16, tag="xTh")     # inputs^T bf16
            x0q8 = pers.tile([128, KU, N], F8, tag="x0q8")     # inputs^T fp8
            x1q8 = pers.tile([128, KU, N], F8, tag="x1q8")
            x2q8 = pers.tile([128, KU, N], F8, tag="x2q8")
            w3x8 = pers.tile([128, KU, N], F8, tag="w3x8")     # (w3*64)*x2^T
            attT8 = pers.tile([128, KU, N], F8, tag="attT8")   # att^T * 8
            xO8 = pers.tile([128, NT, U], F8, tag="xO8")       # row-major x2
            tWh8 = pers.tile([128, H, KU, U], F8, tag="tWh8")  # *32
            cWh8 = pers.tile([128, H, KU, U], F8, tag="cWh8")  # *32
            ffWx = pers.tile([128, KU, U], BF16, tag="ffWx")   # x-half *256
            frWx = pers.tile([128, KU, U], BF16, tag="frWx")
            ffW8 = pers.tile([128, KU, U], F8, tag="ffW8")     # att-half *32
            frW8 = pers.tile([128, KU, U], F8, tag="frW8")
            ffx8 = pers.tile([128, 2, U], F8, tag="ffx8")      # x k0-1 *256
            frx8 = pers.tile([128, 2, U], F8, tag="frx8")
            tbsb = pers.tile([128, H, KU], F32, tag="tbsb")
            cbsb = pers.tile([128, H, KU], F32, tag="cbsb")
            awsb = pers.tile([128, 12], F32, tag="awsb")       # w1|w2|w3 cols
            w1h8 = pers.tile([128, KU, 16], F8, tag="w1h8")    # *64, col 0
            w2h8 = pers.tile([128, KU, 16], F8, tag="w2h8")    # *64, col 0
            aw3s = pers.tile([128, KU], F32, tag="aw3s")       # w3 * 64 f32
            ab_sb = pers.tile([1, 1], F32, tag="ab_sb")
            nab_sb = pers.tile([1, 1], F32, tag="nab_sb")
            ffb_h = pers.tile([1, U], BF16, tag="ffb_h")       # *256
            frb_h = pers.tile([1, U], BF16, tag="frb_h")       # *256
            thr = pers.tile([1, N], BF16, tag="thr")   # exp(-(s1+ab))
            s2f = pers.tile([128, NT], F32, tag="s2f")
            ones_row = pers.tile([1, 128], BF16, tag="ones_row")
            ones2c8 = pers.tile([128, 32], F8, tag="ones2c8")  # DR ones pairs
            identb = pers.tile([128, 128], BF16, tag="identb")
            ident8 = pers.tile([128, 128], F8, tag="ident8")
            identf = pers.tile([128, 128], F32, tag="identf")

            nc.vector.memset(ones_row, 1.0)
            nc.vector.memset(ones2c8, 1.0)
            make_identity(nc, identb)
            make_identity(nc, ident8)
            make_identity(nc, identf)

            # ===== Phase A + highway layer 0, merged ======================
            # l0 slabs are emitted as soon as their token group and weights
            # land, so l0's matmuls run during the tg2/tg3 DMA stream
            # instead of as a separate dense block afterwards.
            with tc.tile_pool(name="stg", bufs=2) as stg, \
                 tc.tile_pool(name="stgw", bufs=2) as stgw, \
                 tc.tile_pool(name="hb0", bufs=2, space="PSUM") as hb0, \
                 tc.tile_pool(name="hs0", bufs=3) as hs0, \
                 tc.tile_pool(name="ptA", bufs=1, space="PSUM") as ptA:

                def keep_warm(n, who):
                    warmp = ptA.tile([128, 512], F32, tag="ptk0",
                                     name=f"warm_{who}")
                    for i in range(n):
                        nc.tensor.matmul(warmp[:, 0:128], identb, identb,
                                         start=True, stop=True)

                def l0_slab(t):
                    nsl = slice(t * 512, (t + 1) * 512)
                    for m in range(KU):
                        msl = slice(m * 128, (m + 1) * 128)
                        pt = hb0.tile([128, 512], F32, tag="pt")
                        pc = hb0.tile([128, 512], F32, tag="pc")
                        for kk in range(KU // 2):
                            nc.tensor.matmul(
                                pt, pair(tWh8[:, 0], kk, msl),
                                pair(x0q8, kk, nsl), perf_mode=DR,
                                start=(kk == 0), stop=(kk == 1))
                        for kk in range(KU // 2):
                            nc.tensor.matmul(
                                pc, pair(cWh8[:, 0], kk, msl),
                                pair(x0q8, kk, nsl), perf_mode=DR,
                                start=(kk == 0), stop=(kk == 1))
                        th = hs0.tile([128, 512], BF16, tag="th")
                        ch = hs0.tile([128, 512], BF16, tag="ch")
                        nc.scalar.activation(
                            th, pt, AF.Relu, bias=tbsb[:, 0, m:m + 1],
                            scale=1.0 / WSH)
                        nc.scalar.activation(
                            ch, pc, AF.Sigmoid, bias=cbsb[:, 0, m:m + 1],
                            scale=1.0 / WSH)
                        dh = hs0.tile([128, 512], BF16, tag="dh")
                        nc.vector.tensor_tensor(
                            dh, th, x0q8[:, m, nsl], op=OP.subtract)
                        mh = hs0.tile([128, 512], BF16, tag="mh")
                        nc.vector.tensor_tensor(mh, ch, dh, op=OP.mult)
                        nc.gpsimd.tensor_tensor(
                            x1q8[:, m, nsl], x0q8[:, m, nsl], mh,
                            op=OP.add)

                # highway-weight loads interleaved after tg0/tg1 so layer-0
                # can start as soon as the first column group lands; one
                # 1MB DMA + one wide cast per (layer, gate)
                def emit_weights(l, wi):
                    wsrc, wdst = ((tW, tWh8), (cW, cWh8))[wi]
                    wv = wsrc[l].rearrange("(k p) m -> p k m", p=128)
                    ws = stgw.tile([128, KU, U], F32, tag="ws",
                                   name=f"ws_{l}_{wi}")
                    nc.sync.dma_start(ws, wv)
                    if wi == 0:
                        nc.vector.tensor_scalar_mul(wdst[:, l], ws, WSH)
                    else:
                        nc.scalar.mul(wdst[:, l], ws, WSH)

                # warm the PE HAM clock-gate during the initial DMA wait
                keep_warm(32, "init")
                for tg in range(NS):
                    # one 1MB DMA per 512-token group, straight into the
                    # persistent row-major copy (reused by phase E)
                    nc.sync.dma_start(
                        x0row[:, 4 * tg:4 * tg + 4, :],
                        x_in.flatten_outer_dims().rearrange(
                            "(t p) u -> p t u", p=128)[:, 4 * tg:4 * tg + 4])
                    ptk = [ptA.tile([128, 512], F32, tag=f"ptk{k}",
                                    name=f"ptk_{tg}_{k}")
                           for k in range(KU)]
                    for tt in range(4):
                        t = tg * 4 + tt
                        for k in range(KU):
                            nc.tensor.transpose(
                                ptk[k][:, tt * 128:(tt + 1) * 128],
                                x0row[:, t, k * 128:(k + 1) * 128], identf)
                    for k in range(KU):
                        sl = slice(tg * 512, (tg + 1) * 512)
                        if k % 2 == 0:
                            nc.vector.tensor_copy(xTh[:, k, sl], ptk[k])
                            nc.scalar.copy(x0q8[:, k, sl], ptk[k])
                        else:
                            nc.scalar.copy(xTh[:, k, sl], ptk[k])
                            nc.vector.tensor_copy(x0q8[:, k, sl], ptk[k])
                    keep_warm(6, f"tg{tg}")
                    if tg == 0:
                        nc.sync.dma_start(
                            tbsb, tb.rearrange("l (m p) -> p l m", p=128))
                        nc.sync.dma_start(
                            cbsb, cb.rearrange("l (m p) -> p l m", p=128))
                        nc.sync.dma_start(
                            awsb, aW.rearrange("(w m p) -> p (w m)",
                                               p=128, w=3))
                        for k in range(KU):
                            nc.vector.tensor_scalar_mul(
                                w1h8[:, k, 0:1], awsb[:, k:k + 1], WSA)
                            nc.vector.tensor_scalar_mul(
                                w2h8[:, k, 0:1], awsb[:, KU + k:KU + k + 1],
                                WSA)
                            nc.scalar.mul(aw3s[:, k:k + 1],
                                          awsb[:, 8 + k:9 + k], WSA)
                        nc.sync.dma_start(ab_sb, ab[None, :])
                        nc.scalar.mul(nab_sb, ab_sb, -1.0)
                        fb = stg.tile([1, U], F32, tag="fb")
                        nc.sync.dma_start(fb, ffb[None, :])
                        nc.vector.tensor_scalar_mul(ffb_h, fb, WSF)
                        fb2 = stg.tile([1, U], F32, tag="fb")
                        nc.sync.dma_start(fb2, frb[None, :])
                        nc.vector.tensor_scalar_mul(frb_h, fb2, WSF)
                        emit_weights(0, 0)
                    elif tg == 1:
                        emit_weights(0, 1)
                        l0_slab(0)
                    elif tg == H:
                        emit_weights(1, 0)
                        l0_slab(1)
                    else:
                        emit_weights(1, 1)
                        l0_slab(2)
                        l0_slab(3)

            # ===== Phase B layer 0: wide [128,1024] 2-bank psum tiles ======
            with tc.tile_pool(name="hw0", bufs=2, space="PSUM") as hw0, \
                 tc.tile_pool(name="hs0", bufs=3) as hs0:
                for tp in range(NS // 2):              # 1024-token slabs
                    wsl = slice(tp * 1024, (tp + 1) * 1024)
                    for m in range(KU):
                        msl = slice(m * 128, (m + 1) * 128)
                        pt = hw0.tile([128, 1024], F32, tag="pt")
                        pc = hw0.tile([128, 1024], F32, tag="pc")
                        for h2 in range(2):
                            nsl = slice(tp * 1024 + h2 * 512,
                                        tp * 1024 + (h2 + 1) * 512)
                            psl = slice(h2 * 512, (h2 + 1) * 512)
                            for kk in range(KU // 2):
                                nc.tensor.matmul(
                                    pt[:, psl], pair(tWh8[:, 0], kk, msl),
                                    pair(x0q8, kk, nsl), perf_mode=DR,
                                    start=(kk == 0), stop=(kk == 1))
                            for kk in range(KU // 2):
                                nc.tensor.matmul(
                                    pc[:, psl], pair(cWh8[:, 0], kk, msl),
                                    pair(x0q8, kk, nsl), perf_mode=DR,
                                    start=(kk == 0), stop=(kk == 1))
                        th = hs0.tile([128, 1024], BF16, tag="th")
                        ch = hs0.tile([128, 1024], BF16, tag="ch")
                        nc.scalar.activation(
                            th, pt, AF.Relu, bias=tbsb[:, 0, m:m + 1],
                            scale=1.0 / WSH)
                        nc.scalar.activation(
                            ch, pc, AF.Sigmoid, bias=cbsb[:, 0, m:m + 1],
                            scale=1.0 / WSH)
                        dh = hs0.tile([128, 1024], BF16, tag="dh")
                        nc.vector.tensor_tensor(
                            dh, th, x0q8[:, m, wsl], op=OP.subtract)
                        mh = hs0.tile([128, 1024], BF16, tag="mh")
                        nc.vector.tensor_tensor(mh, ch, dh, op=OP.mult)
                        nc.gpsimd.tensor_tensor(
                            x1q8[:, m, wsl], x0q8[:, m, wsl], mh, op=OP.add)

            # ===== Phase B layer 1 + C: highway + per-slab attention prep ==
            # During layer 1, each finished 1024-token slab immediately gets
            # its row-major transposes, w3x, s1/thr and s2 emitted, filling
            # PE slack in the elementwise-bound highway stretch.
            with tc.tile_pool(name="hwp", bufs=2, space="PSUM") as hwp, \
                 tc.tile_pool(name="pcp", bufs=2, space="PSUM") as pcp, \
                 tc.tile_pool(name="pcp1", bufs=1, space="PSUM") as pcp1, \
                 tc.tile_pool(name="hws", bufs=4) as hws:
                s2p = pcp1.tile([128, NT], F32, tag="s2p")

                def prep_slab(tp):
                    """attention prep for tokens [tp*1024, (tp+1)*1024)."""
                    for k in range(KU):
                        wsl = slice(tp * 1024, (tp + 1) * 1024)
                        nc.vector.tensor_scalar_mul(
                            w3x8[:, k, wsl], x2q8[:, k, wsl],
                            aw3s[:, k:k + 1])
                    for jt in range(8 * tp, 8 * tp + 8):
                        # full-bank staging tile so rotating bufs land in
                        # different banks (PE-write vs DVE-read collision)
                        ptr = pcp.tile([128, 2048], F8, tag="ptr")
                        ptv = ptr[:, 0:1024].rearrange(
                            "p (n two) -> p n two", two=2)
                        for k in range(KU):
                            nc.tensor.transpose(
                                ptv[:, k * 128:(k + 1) * 128, 0:1],
                                x2q8[:, k, jt * 128:(jt + 1) * 128], ident8)
                        if jt % 2 == 0:
                            nc.vector.tensor_copy(xO8[:, jt, :],
                                                  ptv[:, :, 0:1])
                        else:
                            nc.scalar.copy(xO8[:, jt, :], ptv[:, :, 0:1])
                    for t in (2 * tp, 2 * tp + 1):
                        ps1 = pcp1.tile([1, 512], F32, tag="ps1")
                        for kk in range(KU // 2):
                            nc.tensor.matmul(
                                ps1, pair(w1h8, kk, slice(0, 1)),
                                pair(x2q8, kk,
                                     slice(t * 512, (t + 1) * 512)),
                                perf_mode=DR,
                                start=(kk == 0), stop=(kk == 1))
                        nc.scalar.activation(
                            thr[:, t * 512:(t + 1) * 512], ps1, AF.Exp,
                            bias=nab_sb, scale=-1.0 / WSA)
                    for jt in range(8 * tp, 8 * tp + 8):
                        jsl = slice(jt * 128, (jt + 1) * 128)
                        for kk in range(KU // 2):
                            nc.tensor.matmul(
                                s2p[:, jt:jt + 1],
                                pair(x2q8, kk, jsl),
                                pair(w2h8, kk, slice(0, 1)),
                                perf_mode=DR,
                                start=(kk == 0), stop=(kk == 1))
                    nc.scalar.mul(s2f[:, 8 * tp:8 * tp + 8],
                                  s2p[:, 8 * tp:8 * tp + 8], 1.0 / WSA)

                for t in range(NS):                    # 512-token slabs
                    nsl = slice(t * 512, (t + 1) * 512)
                    for m in range(KU):
                        if t == 0:
                            # keep the PE HAM clock-gate warm through the
                            # l0-tail / l1-ramp valley (prep pool is idle)
                            wt = pcp.tile([128, 2048], F8, tag="ptr",
                                          name=f"wl1_{m}").bitcast(F32)
                            for i in range(4):
                                nc.tensor.matmul(wt[:, 0:128], identb,
                                                 identb, start=True,
                                                 stop=True)
                        msl = slice(m * 128, (m + 1) * 128)
                        pt = hwp.tile([128, 512], F32, tag="pt")
                        pc = hwp.tile([128, 512], F32, tag="pc")
                        for kk in range(KU // 2):
                            nc.tensor.matmul(
                                pt, pair(tWh8[:, 1], kk, msl),
                                pair(x1q8, kk, nsl), perf_mode=DR,
                                start=(kk == 0), stop=(kk == 1))
                        for kk in range(KU // 2):
                            nc.tensor.matmul(
                                pc, pair(cWh8[:, 1], kk, msl),
                                pair(x1q8, kk, nsl), perf_mode=DR,
                                start=(kk == 0), stop=(kk == 1))
                        th = hws.tile([128, 512], BF16, tag="th")
                        ch = hws.tile([128, 512], BF16, tag="ch")
                        nc.scalar.activation(
                            th, pt, AF.Relu, bias=tbsb[:, 1, m:m + 1],
                            scale=1.0 / WSH)
                        nc.scalar.activation(
                            ch, pc, AF.Sigmoid, bias=cbsb[:, 1, m:m + 1],
                            scale=1.0 / WSH)
                        dh = hws.tile([128, 512], BF16, tag="dh")
                        nc.vector.tensor_tensor(
                            dh, th, x1q8[:, m, nsl], op=OP.subtract)
                        mh = hws.tile([128, 512], BF16, tag="mh")
                        nc.vector.tensor_tensor(
                            mh, ch, dh, op=OP.mult)
                        nc.gpsimd.tensor_tensor(
                            x2q8[:, m, nsl], x1q8[:, m, nsl], mh,
                            op=OP.add)
                    if t % 2 == 1:
                        prep_slab(t // 2)

            # ============= Phase D: pairwise softmax attention =============
            fWv = ffW.rearrange("(k p) m -> k p m", p=128)
            rWv = frW.rearrange("(k p) m -> k p m", p=128)
            # x-half (k 0..3) -> bf16 *256 ; att-half (k 4..7) -> fp8 *32
            fuse_chunks = ([(fWv, ffWx, ffW8, k) for k in range(2 * KU)] +
                           [(rWv, frWx, frW8, k) for k in range(2 * KU)])
            with tc.tile_pool(name="pdn", bufs=2, space="PSUM") as pdn, \
                 tc.tile_pool(name="pds", bufs=2, space="PSUM") as pds, \
                 tc.tile_pool(name="pdr", bufs=1, space="PSUM") as pdr, \
                 tc.tile_pool(name="pbc", bufs=1, space="PSUM") as pbc, \
                 tc.tile_pool(name="stgf", bufs=4) as stgf, \
                 tc.tile_pool(name="dsb", bufs=5) as dsb, \
                 tc.tile_pool(name="ehp", bufs=8) as ehp:
                ones2v = ones2c8.rearrange("p (two s) -> p two s", two=2)
                for b in range(BPC):
                    for h in range(IH):
                        # drip-feed fuse-gate weight loads (DMA idle here)
                        unit = b * IH + h
                        for ci in range(unit * 4, unit * 4 + 4):
                            wv_, wbf_, w8_, k_ = fuse_chunks[ci]
                            wsf = stgf.tile([128, U], F32, tag="wsf",
                                            name=f"wsf_{ci}")
                            nc.sync.dma_start(wsf, wv_[k_])
                            if k_ < KU:
                                if ci % 2 == 0:
                                    nc.vector.tensor_scalar_mul(
                                        wbf_[:, k_, :], wsf, WSF)
                                else:
                                    nc.scalar.mul(wbf_[:, k_, :], wsf, WSF)
                                if k_ < 2:   # fp8 copy for the DR x-half
                                    x8d = ffx8 if ci < 2 * KU else frx8
                                    nc.scalar.copy(x8d[:, k_, :],
                                                   wbf_[:, k_, :])
                            else:
                                if ci % 2 == 0:
                                    nc.vector.tensor_scalar_mul(
                                        w8_[:, k_ - KU, :], wsf, WSH)
                                else:
                                    nc.scalar.mul(w8_[:, k_ - KU, :], wsf,
                                                  WSH)
                        isl = slice(b * L + h * 512, b * L + (h + 1) * 512)
                        # keep-warm burst across the unit boundary
                        wtd = pbc.tile([128, 512], F32, tag="pb",
                                       name=f"wd_{b}_{h}")
                        for i in range(4):
                            nc.tensor.matmul(wtd[:, 0:128], identb, identb,
                                             start=True, stop=True)
                        thbc = dsb.tile([128, 512], BF16, tag="thbc")
                        pb1 = pbc.tile([128, 512], F32, tag="pb",
                                       name=f"pb1_{b}_{h}")
                        nc.tensor.matmul(pb1, ones_row, thr[:, isl],
                                         start=True, stop=True)
                        nc.scalar.copy(thbc, pb1)
                        # ---- stage 1: scores -> exp -> fp8 eh pair tiles
                        ehs = []
                        for p in range(JT // 2):      # j-tile pairs
                            jg = b * JT + 2 * p
                            ps = pds.tile([128, 1024], F32, tag="ps",
                                          name=f"ps_{b}_{h}_{p}")
                            ehb = ehp.tile([128, 1024], F8, tag="ehb",
                                           name=f"ehb_{b}_{h}_{p}")
                            ehbf = dsb.tile([128, 1024], BF16, tag="ehbf")
                            for half in range(2):
                                jsl = slice((jg + half) * 128,
                                            (jg + half + 1) * 128)
                                hsl = slice(half * 512, (half + 1) * 512)
                                for kk in range(KU // 2):
                                    nc.tensor.matmul(
                                        ps[:, hsl], pair(w3x8, kk, jsl),
                                        pair(x2q8, kk, isl), perf_mode=DR,
                                        start=(kk == 0), stop=(kk == 1))
                                nc.scalar.activation(
                                    ehbf[:, hsl], ps[:, hsl], AF.Exp,
                                    bias=s2f[:, jg + half:jg + half + 1],
                                    scale=1.0 / WSA)
                                nc.vector.tensor_tensor(
                                    ehb[:, hsl], ehbf[:, hsl], thbc,
                                    op=OP.max)
                            ehs.append(
                                ehb.rearrange("p (two n) -> p two n", two=2))
                        # ---- stage 2: denominator first, then du-major
                        # numerator accumulation
                        pr = pdr.tile([1, 512], F32, tag="pr")
                        for p in range(JT // 2):
                            nc.tensor.matmul(
                                pr, ones2v[:, :, 0:1], ehs[p], perf_mode=DR,
                                start=(p == 0), stop=(p == JT // 2 - 1))
                        rec = dsb.tile([1, 512], F32, tag="rec")
                        nc.vector.reciprocal_approx_fast(rec, pr)
                        rech = dsb.tile([1, 512], BF16, tag="rech")
                        nc.scalar.mul(rech, rec, ATS)
                        rbc = dsb.tile([128, 512], BF16, tag="rbc")
                        pb2 = pbc.tile([128, 512], F32, tag="pb",
                                       name=f"pb2_{b}_{h}")
                        nc.tensor.matmul(pb2, ones_row, rech,
                                         start=True, stop=True)
                        nc.scalar.copy(rbc, pb2)
                        for du in range(KU):
                            pn = pdn.tile([128, 512], F32, tag="pn",
                                          name=f"pn_{b}_{h}_{du}")
                            for p in range(JT // 2):
                                jg = b * JT + 2 * p
                                nc.tensor.matmul(
                                    pn,
                                    xO8[:, jg:jg + 2,
                                        du * 128:(du + 1) * 128],
                                    ehs[p], perf_mode=DR,
                                    start=(p == 0), stop=(p == JT // 2 - 1))
                            # drain + normalize (*8) in one pass
                            nc.vector.tensor_tensor(
                                attT8[:, du, isl], pn, rbc, op=OP.mult)

            # ============= Phase E: fuse gates + output ====================
            with tc.tile_pool(name="pep", bufs=2, space="PSUM") as pep, \
                 tc.tile_pool(name="peb", bufs=1, space="PSUM") as peb, \
                 tc.tile_pool(name="esb", bufs=3) as esb:
                # broadcast fuse biases (*256) to [128, 512] once
                fbb = esb.tile([128, U], BF16, tag="fbb")
                rbb = esb.tile([128, U], BF16, tag="rbb")
                pfb = peb.tile([128, 512], F32, tag="pfb", name="pfb_f")
                nc.tensor.matmul(pfb, ones_row, ffb_h, start=True, stop=True)
                nc.vector.tensor_copy(fbb, pfb)
                prb = peb.tile([128, 512], F32, tag="pfb", name="pfb_r")
                nc.tensor.matmul(prb, ones_row, frb_h, start=True, stop=True)
                nc.vector.tensor_copy(rbb, prb)
                for mt in range(NT):
                    msl = slice(mt * 128, (mt + 1) * 128)
                    pz = pep.tile([128, 512], F32, tag="pz")
                    pr2 = pep.tile([128, 512], F32, tag="pr2")
                    # x-half: k-tiles 0-1 as one fp8 DR pass, 2-3 bf16
                    nc.tensor.matmul(pz, pair(x0q8, 0, msl), ffx8,
                                     perf_mode=DR, start=True, stop=False)
                    nc.tensor.matmul(pr2, pair(x0q8, 0, msl), frx8,
                                     perf_mode=DR, start=True, stop=False)
                    for k in (2, 3):
                        nc.tensor.matmul(pz, xTh[:, k, msl], ffWx[:, k, :],
                                         start=False, stop=False)
                        nc.tensor.matmul(pr2, xTh[:, k, msl], frWx[:, k, :],
                                         start=False, stop=False)
                    for kk in range(KU // 2):    # att-half, fp8 DR
                        nc.tensor.matmul(pz, pair(attT8, kk, msl),
                                         pair(ffW8, kk), perf_mode=DR,
                                         start=False, stop=(kk == 1))
                        nc.tensor.matmul(pr2, pair(attT8, kk, msl),
                                         pair(frW8, kk), perf_mode=DR,
                                         start=False, stop=(kk == 1))
                    # bias add on vector (frees psum early), sigmoid on
                    # scalar from SBUF
                    pzs = esb.tile([128, U], BF16, tag="pzs")
                    prs = esb.tile([128, U], BF16, tag="prs")
                    nc.vector.tensor_tensor(pzs, pz, fbb, op=OP.add)
                    nc.vector.tensor_tensor(prs, pr2, rbb, op=OP.add)
                    zh = esb.tile([128, U], BF16, tag="zh")
                    rh = esb.tile([128, U], BF16, tag="rh")
                    q = esb.tile([128, U], F32, tag="q")
                    p2 = esb.tile([128, U], F32, tag="p2")
                    ot = esb.tile([128, U], F32, tag="ot")
                    x0t = x0row[:, mt, :]
                    if mt == NT - 1:
                        # shorten the kernel tail: split across engines
                        hU = U // 2
                        nc.scalar.activation(zh, pzs, AF.Sigmoid,
                                             scale=1.0 / WSF)
                        nc.scalar.square(q, zh)
                        nc.scalar.activation(rh, prs, AF.Sigmoid,
                                             scale=1.0 / WSF)
                        nc.vector.tensor_tensor(p2[:, :hU], rh[:, :hU],
                                                x0t[:, :hU], op=OP.mult)
                        nc.gpsimd.tensor_tensor(p2[:, hU:], rh[:, hU:],
                                                x0t[:, hU:], op=OP.mult)
                        nc.vector.tensor_tensor(ot[:, :hU], q[:, :hU],
                                                p2[:, :hU], op=OP.add)
                        nc.gpsimd.tensor_tensor(ot[:, hU:], q[:, hU:],
                                                p2[:, hU:], op=OP.add)
                    else:
                        nc.scalar.activation(zh, pzs, AF.Sigmoid,
                                             scale=1.0 / WSF)
                        nc.scalar.activation(rh, prs, AF.Sigmoid,
                                             scale=1.0 / WSF)
                        nc.scalar.square(q, zh)
                        nc.vector.tensor_tensor(p2, rh, x0t, op=OP.mult)
                        nc.gpsimd.tensor_tensor(ot, q, p2, op=OP.add)
                    nc.sync.dma_start(outv[mt], ot)

    nc.compile()
    return nc


_NC_CACHE = None


def _get_nc():
    global _NC_CACHE
    if _NC_CACHE is None:
        _NC_CACHE = build_nc()
    return _NC_CACHE


def kernel(**inputs) -> np.ndarray:
    from concourse.bass_utils import run_bass_kernel_spmd

    nc = _get_nc()
    full = {k: np.ascontiguousarray(np.asarray(v, dtype=np.float32))
            for k, v in inputs.items()}
    in_maps = []
    for c in range(NCORES):
        m = dict(full)
        m["inputs"] = np.ascontiguousarray(
            full["inputs"][c * BPC:(c + 1) * BPC])
        in_maps.append(m)
    res = run_bass_kernel_spmd(nc, in_maps, core_ids=list(range(NCORES)))
    return np.concatenate([res.results[c]["out"] for c in range(NCORES)],
                          axis=0)


# revision 52
# speedup vs baseline: 1.1028x; 1.0193x over previous
"""Trainium2 Bass kernel for nn_Encoding_layer (highway stack + pairwise MLP
attention + fuse gates).

Sharding: data-parallel over batch B=16 across 8 NeuronCores (2 batches per
core); all dense weights replicated. No collectives.

fp8-e4m3 DoubleRow matmuls for the compute-heavy GEMMs, with the
schedule restructured to keep the PE HAM clock-gate warm
(HW: 219.1us vs 262.7us baseline; rel err 3.9e-3 vs 2e-2 budget):
  - DoubleRow contracts 256 rows/pass (2 fp8 weights per PE cell); operand
    pairs are adjacent k-tiles in the free dim of the [128, KU, N] tilings.
  - Quantization (numpy-validated, rel err ~3e-3 vs 2e-2 budget):
      highway (x fp8, W fp8*32)   scores s3 (w3x fp8*64 x x2 fp8)
      att numerator (xO fp8 x eh fp8)   att stored fp8*8
      fuse gates: x-half bf16 (W bf16*256) + att-half fp8 DR (att*8 x W*32)
    All scales undone via scalar.activation(func, scale=2^-k).
  - Highway layer 0 is merged into the load phase: each slab's matmuls are
    emitted as soon as its token group + weights land, so l0 computes
    during the remaining input DMA stream.
  - Attention prep (row-major transposes, s1/s2/thr) is emitted per-slab
    inside highway layer 1, filling PE slack in the elementwise-bound
    highway stretch; keep-warm matmul bursts bridge the known idle
    valleys so HAM stays at K=8/8.
  - Phase D per (b,h) unit is two-staged: (1) all four j-tile-pair score
    blocks -> exp -> fp8 eh pair tiles (SBUF), (2) denominator then
    du-major numerator accumulation, so wide score psum is double-buffered
    within the 8-bank budget.
  - eh pair tiles [128,2,512] fp8 are exactly the DoubleRow moving operand
    of the numerator.  relu-as-clamp: M^T = max(exp(s3+s2), exp(-(s1+ab)))
    (the per-column factor exp(s1+ab) cancels in the softmax).
"""

import numpy as np

B, L, U, H = 16, 1024, 512, 2
NCORES = 8
BPC = B // NCORES          # batches per core
N = BPC * L                # token columns per core
KU = U // 128              # 4  u-tiles
NT = N // 128              # 16 row-tiles per core
NS = N // 512              # 4  512-wide column slices per core
JT = L // 128              # 8  j-tiles per batch
IH = L // 512              # 2  i-halves per batch

WSH = 32.0                 # highway weight prescale (2^5)
WSA = 64.0                 # aW prescale (2^6)
WSF = 256.0                # fuse-gate effective prescale (2^8)
ATS = 8.0                  # att fp8 prescale (2^3)


def build_nc():
    import concourse.bacc as bacc
    import concourse.tile as tile
    from concourse import mybir
    from concourse.masks import make_identity

    F32 = mybir.dt.float32
    BF16 = mybir.dt.bfloat16
    F8 = mybir.dt.float8e4
    AF = mybir.ActivationFunctionType
    OP = mybir.AluOpType
    DR = mybir.MatmulPerfMode.DoubleRow

    nc = bacc.Bacc("TRN2", target_bir_lowering=False, debug=False,
                   num_devices=NCORES)

    x_in = nc.dram_tensor("inputs", [BPC, L, U], F32, kind="ExternalInput").ap()
    tW = nc.dram_tensor("tW", [H, U, U], F32, kind="ExternalInput").ap()
    tb = nc.dram_tensor("tb", [H, U], F32, kind="ExternalInput").ap()
    cW = nc.dram_tensor("cW", [H, U, U], F32, kind="ExternalInput").ap()
    cb = nc.dram_tensor("cb", [H, U], F32, kind="ExternalInput").ap()
    aW = nc.dram_tensor("aW", [3 * U], F32, kind="ExternalInput").ap()
    ab = nc.dram_tensor("ab", [1], F32, kind="ExternalInput").ap()
    frW = nc.dram_tensor("frW", [2 * U, U], F32, kind="ExternalInput").ap()
    frb = nc.dram_tensor("frb", [U], F32, kind="ExternalInput").ap()
    ffW = nc.dram_tensor("ffW", [2 * U, U], F32, kind="ExternalInput").ap()
    ffb = nc.dram_tensor("ffb", [U], F32, kind="ExternalInput").ap()
    out = nc.dram_tensor("out", [BPC, L, U], F32, kind="ExternalOutput").ap()

    xv = x_in.flatten_outer_dims().rearrange("(t p) u -> t p u", p=128)
    outv = out.flatten_outer_dims().rearrange("(t p) u -> t p u", p=128)

    def pair(t, k2, sl=None):
        """[128, 2, *] DoubleRow view of adjacent k-tiles k2*2, k2*2+1."""
        return t[:, 2 * k2:2 * k2 + 2, sl] if sl is not None \
            else t[:, 2 * k2:2 * k2 + 2, :]

    with tile.TileContext(nc) as tc:
        with tc.tile_pool(name="pers", bufs=1) as pers:
            # ---- persistent SBUF tensors ----
            x0row = pers.tile([128, NT, U], F32, tag="x0row")  # inputs row-maj
            xTh = pers.tile([128, KU, N], BF16, tag="xTh")     # inputs^T bf16
            x0q8 = pers.tile([128, KU, N], F8, tag="x0q8")     # inputs^T fp8
            x1q8 = pers.tile([128, KU, N], F8, tag="x1q8")
            x2q8 = pers.tile([128, KU, N], F8, tag="x2q8")
            w3x8 = pers.tile([128, KU, N], F8, tag="w3x8")     # (w3*64)*x2^T
            attT8 = pers.tile([128, KU, N], F8, tag="attT8")   # att^T * 8
            xO8 = pers.tile([128, NT, U], F8, tag="xO8")       # row-major x2
            tWh8 = pers.tile([128, H, KU, U], F8, tag="tWh8")  # *32
            cWh8 = pers.tile([128, H, KU, U], F8, tag="cWh8")  # *32
            ffWx = pers.tile([128, KU, U], BF16, tag="ffWx")   # x-half *256
            frWx = pers.tile([128, KU, U], BF16, tag="frWx")
            ffW8 = pers.tile([128, KU, U], F8, tag="ffW8")     # att-half *32
            frW8 = pers.tile([128, KU, U], F8, tag="frW8")
            ffx8 = pers.tile([128, 2, U], F8, tag="ffx8")      # x k0-1 *256
            frx8 = pers.tile([128, 2, U], F8, tag="frx8")
            tbsb = pers.tile([128, H, KU], F32, tag="tbsb")
            cbsb = pers.tile([128, H, KU], F32, tag="cbsb")
            awsb = pers.tile([128, 12], F32, tag="awsb")       # w1|w2|w3 cols
            w1h8 = pers.tile([128, KU, 16], F8, tag="w1h8")    # *64, col 0
            w2h8 = pers.tile([128, KU, 16], F8, tag="w2h8")    # *64, col 0
            aw3s = pers.tile([128, KU], F32, tag="aw3s")       # w3 * 64 f32
            ab_sb = pers.tile([1, 1], F32, tag="ab_sb")
            nab_sb = pers.tile([1, 1], F32, tag="nab_sb")
            ffb_h = pers.tile([1, U], BF16, tag="ffb_h")       # *256
            frb_h = pers.tile([1, U], BF16, tag="frb_h")       # *256
            thr = pers.tile([1, N], BF16, tag="thr")   # exp(-(s1+ab))
            s2f = pers.tile([128, NT], F32, tag="s2f")
            ones_row = pers.tile([1, 128], BF16, tag="ones_row")
            ones2c8 = pers.tile([128, 32], F8, tag="ones2c8")  # DR ones pairs
            identb = pers.tile([128, 128], BF16, tag="identb")
            ident8 = pers.tile([128, 128], F8, tag="ident8")
            identf = pers.tile([128, 128], F32, tag="identf")

            nc.vector.memset(ones_row, 1.0)
            nc.vector.memset(ones2c8, 1.0)
            make_identity(nc, identb)
            make_identity(nc, ident8)
            make_identity(nc, identf)

            # ===== Phase A + highway layer 0, merged ======================
            # l0 slabs are emitted as soon as their token group and weights
            # land, so l0's matmuls run during the tg2/tg3 DMA stream
            # instead of as a separate dense block afterwards.
            with tc.tile_pool(name="stg", bufs=2) as stg, \
                 tc.tile_pool(name="stgw", bufs=2) as stgw, \
                 tc.tile_pool(name="hb0", bufs=2, space="PSUM") as hb0, \
                 tc.tile_pool(name="hs0", bufs=3) as hs0, \
                 tc.tile_pool(name="ptA", bufs=1, space="PSUM") as ptA:

                def keep_warm(n, who):
                    warmp = ptA.tile([128, 512], F32, tag="ptk0",
                                     name=f"warm_{who}")
                    for i in range(n):
                        nc.tensor.matmul(warmp[:, 0:128], identb, identb,
                                         start=True, stop=True)

                def l0_slab(t):
                    nsl = slice(t * 512, (t + 1) * 512)
                    for m in range(KU):
                        msl = slice(m * 128, (m + 1) * 128)
                        pt = hb0.tile([128, 512], F32, tag="pt")
                        pc = hb0.tile([128, 512], F32, tag="pc")
                        for kk in range(KU // 2):
                            nc.tensor.matmul(
                                pt, pair(tWh8[:, 0], kk, msl),
                                pair(x0q8, kk, nsl), perf_mode=DR,
                                start=(kk == 0), stop=(kk == 1))
                        for kk in range(KU // 2):
                            nc.tensor.matmul(
                                pc, pair(cWh8[:, 0], kk, msl),
                                pair(x0q8, kk, nsl), perf_mode=DR,
                                start=(kk == 0), stop=(kk == 1))
                        th = hs0.tile([128, 512], BF16, tag="th")
                        ch = hs0.tile([128, 512], BF16, tag="ch")
                        nc.scalar.activation(
                            th, pt, AF.Relu, bias=tbsb[:, 0, m:m + 1],
                            scale=1.0 / WSH)
                        nc.scalar.activation(
                            ch, pc, AF.Sigmoid, bias=cbsb[:, 0, m:m + 1],
                            scale=1.0 / WSH)
                        dh = hs0.tile([128, 512], BF16, tag="dh")
                        nc.vector.tensor_tensor(
                            dh, th, x0q8[:, m, nsl], op=OP.subtract)
                        mh = hs0.tile([128, 512], BF16, tag="mh")
                        nc.vector.tensor_tensor(mh, ch, dh, op=OP.mult)
                        nc.gpsimd.tensor_tensor(
                            x1q8[:, m, nsl], x0q8[:, m, nsl], mh,
                            op=OP.add)

                # highway-weight loads interleaved after tg0/tg1 so layer-0
                # can start as soon as the first column group lands; one
                # 1MB DMA + one wide cast per (layer, gate)
                def emit_weights(l, wi):
                    wsrc, wdst = ((tW, tWh8), (cW, cWh8))[wi]
                    wv = wsrc[l].rearrange("(k p) m -> p k m", p=128)
                    ws = stgw.tile([128, KU, U], F32, tag="ws",
                                   name=f"ws_{l}_{wi}")
                    nc.sync.dma_start(ws, wv)
                    if wi == 0:
                        nc.vector.tensor_scalar_mul(wdst[:, l], ws, WSH)
                    else:
                        nc.scalar.mul(wdst[:, l], ws, WSH)

                # warm the PE HAM clock-gate during the initial DMA wait
                keep_warm(32, "init")
                for tg in range(NS):
                    # one 1MB DMA per 512-token group, straight into the
                    # persistent row-major copy (reused by phase E)
                    nc.sync.dma_start(
                        x0row[:, 4 * tg:4 * tg + 4, :],
                        x_in.flatten_outer_dims().rearrange(
                            "(t p) u -> p t u", p=128)[:, 4 * tg:4 * tg + 4])
                    ptk = [ptA.tile([128, 512], F32, tag=f"ptk{k}",
                                    name=f"ptk_{tg}_{k}")
                           for k in range(KU)]
                    for tt in range(4):
                        t = tg * 4 + tt
                        for k in range(KU):
                            nc.tensor.transpose(
                                ptk[k][:, tt * 128:(tt + 1) * 128],
                                x0row[:, t, k * 128:(k + 1) * 128], identf)
                    for k in range(KU):
                        sl = slice(tg * 512, (tg + 1) * 512)
                        if k % 2 == 0:
                            nc.vector.tensor_copy(xTh[:, k, sl], ptk[k])
                            nc.scalar.copy(x0q8[:, k, sl], ptk[k])
                        else:
                            nc.scalar.copy(xTh[:, k, sl], ptk[k])
                            nc.vector.tensor_copy(x0q8[:, k, sl], ptk[k])
                    keep_warm(6, f"tg{tg}")
                    if tg == 0:
                        nc.sync.dma_start(
                            tbsb, tb.rearrange("l (m p) -> p l m", p=128))
                        nc.sync.dma_start(
                            cbsb, cb.rearrange("l (m p) -> p l m", p=128))
                        nc.sync.dma_start(
                            awsb, aW.rearrange("(w m p) -> p (w m)",
                                               p=128, w=3))
                        for k in range(KU):
                            nc.vector.tensor_scalar_mul(
                                w1h8[:, k, 0:1], awsb[:, k:k + 1], WSA)
                            nc.vector.tensor_scalar_mul(
                                w2h8[:, k, 0:1], awsb[:, KU + k:KU + k + 1],
                                WSA)
                            nc.scalar.mul(aw3s[:, k:k + 1],
                                          awsb[:, 8 + k:9 + k], WSA)
                        nc.sync.dma_start(ab_sb, ab[None, :])
                        nc.scalar.mul(nab_sb, ab_sb, -1.0)
                        fb = stg.tile([1, U], F32, tag="fb")
                        nc.sync.dma_start(fb, ffb[None, :])
                        nc.vector.tensor_scalar_mul(ffb_h, fb, WSF)
                        fb2 = stg.tile([1, U], F32, tag="fb")
                        nc.sync.dma_start(fb2, frb[None, :])
                        nc.vector.tensor_scalar_mul(frb_h, fb2, WSF)
                        emit_weights(0, 0)
                    elif tg == 1:
                        emit_weights(0, 1)
                        l0_slab(0)
                    elif tg == H:
                        emit_weights(1, 0)
                        l0_slab(1)
                    else:
                        emit_weights(1, 1)
                        l0_slab(2)
                        l0_slab(3)

            # ===== Phase B layer 0: wide [128,1024] 2-bank psum tiles ======
            with tc.tile_pool(name="hw0", bufs=2, space="PSUM") as hw0, \
                 tc.tile_pool(name="hs0", bufs=3) as hs0:
                for tp in range(NS // 2):              # 1024-token slabs
                    wsl = slice(tp * 1024, (tp + 1) * 1024)
                    for m in range(KU):
                        msl = slice(m * 128, (m + 1) * 128)
                        pt = hw0.tile([128, 1024], F32, tag="pt")
                        pc = hw0.tile([128, 1024], F32, tag="pc")
                        for h2 in range(2):
                            nsl = slice(tp * 1024 + h2 * 512,
                                        tp * 1024 + (h2 + 1) * 512)
                            psl = slice(h2 * 512, (h2 + 1) * 512)
                            for kk in range(KU // 2):
                                nc.tensor.matmul(
                                    pt[:, psl], pair(tWh8[:, 0], kk, msl),
                                    pair(x0q8, kk, nsl), perf_mode=DR,
                                    start=(kk == 0), stop=(kk == 1))
                            for kk in range(KU // 2):
                                nc.tensor.matmul(
                                    pc[:, psl], pair(cWh8[:, 0], kk, msl),
                                    pair(x0q8, kk, nsl), perf_mode=DR,
                                    start=(kk == 0), stop=(kk == 1))
                        th = hs0.tile([128, 1024], BF16, tag="th")
                        ch = hs0.tile([128, 1024], BF16, tag="ch")
                        nc.scalar.activation(
                            th, pt, AF.Relu, bias=tbsb[:, 0, m:m + 1],
                            scale=1.0 / WSH)
                        nc.scalar.activation(
                            ch, pc, AF.Sigmoid, bias=cbsb[:, 0, m:m + 1],
                            scale=1.0 / WSH)
                        dh = hs0.tile([128, 1024], BF16, tag="dh")
                        nc.vector.tensor_tensor(
                            dh, th, x0q8[:, m, wsl], op=OP.subtract)
                        mh = hs0.tile([128, 1024], BF16, tag="mh")
                        nc.vector.tensor_tensor(mh, ch, dh, op=OP.mult)
                        nc.gpsimd.tensor_tensor(
                            x1q8[:, m, wsl], x0q8[:, m, wsl], mh, op=OP.add)

            # ===== Phase B layer 1: wide [128,1024] 2-bank psum tiles =====
            with tc.tile_pool(name="hwp", bufs=2, space="PSUM") as hwp, \
                 tc.tile_pool(name="hws", bufs=4) as hws:
                for tp in range(NS // 2):              # 1024-token slabs
                    wsl = slice(tp * 1024, (tp + 1) * 1024)
                    for m in range(KU):
                        if tp == 0:
                            # keep HAM warm through the l0-tail/l1-ramp
                            wt = hwp.tile([128, 1024], F32, tag="pt",
                                          name=f"wl1_{m}")
                            for i in range(4):
                                nc.tensor.matmul(wt[:, 0:128], identb,
                                                 identb, start=True,
                                                 stop=True)
                        msl = slice(m * 128, (m + 1) * 128)
                        pt = hwp.tile([128, 1024], F32, tag="pt")
                        pc = hwp.tile([128, 1024], F32, tag="pc")
                        for h2 in range(2):
                            nsl = slice(tp * 1024 + h2 * 512,
                                        tp * 1024 + (h2 + 1) * 512)
                            psl = slice(h2 * 512, (h2 + 1) * 512)
                            for kk in range(KU // 2):
                                nc.tensor.matmul(
                                    pt[:, psl], pair(tWh8[:, 1], kk, msl),
                                    pair(x1q8, kk, nsl), perf_mode=DR,
                                    start=(kk == 0), stop=(kk == 1))
                            for kk in range(KU // 2):
                                nc.tensor.matmul(
                                    pc[:, psl], pair(cWh8[:, 1], kk, msl),
                                    pair(x1q8, kk, nsl), perf_mode=DR,
                                    start=(kk == 0), stop=(kk == 1))
                        th = hws.tile([128, 1024], BF16, tag="th")
                        ch = hws.tile([128, 1024], BF16, tag="ch")
                        nc.scalar.activation(
                            th, pt, AF.Relu, bias=tbsb[:, 1, m:m + 1],
                            scale=1.0 / WSH)
                        nc.scalar.activation(
                            ch, pc, AF.Sigmoid, bias=cbsb[:, 1, m:m + 1],
                            scale=1.0 / WSH)
                        dh = hws.tile([128, 1024], BF16, tag="dh")
                        nc.vector.tensor_tensor(
                            dh, th, x1q8[:, m, wsl], op=OP.subtract)
                        mh = hws.tile([128, 1024], BF16, tag="mh")
                        nc.vector.tensor_tensor(mh, ch, dh, op=OP.mult)
                        nc.gpsimd.tensor_tensor(
                            x2q8[:, m, wsl], x1q8[:, m, wsl], mh,
                            op=OP.add)

            # ============= Phase D: pairwise softmax attention =============
            fWv = ffW.rearrange("(k p) m -> k p m", p=128)
            rWv = frW.rearrange("(k p) m -> k p m", p=128)
            # x-half (k 0..3) -> bf16 *256 ; att-half (k 4..7) -> fp8 *32
            fuse_chunks = ([(fWv, ffWx, ffW8, k) for k in range(2 * KU)] +
                           [(rWv, frWx, frW8, k) for k in range(2 * KU)])
            with tc.tile_pool(name="pdn", bufs=2, space="PSUM") as pdn, \
                 tc.tile_pool(name="pds", bufs=2, space="PSUM") as pds, \
                 tc.tile_pool(name="pdr", bufs=1, space="PSUM") as pdr, \
                 tc.tile_pool(name="pbc", bufs=1, space="PSUM") as pbc, \
                 tc.tile_pool(name="stgf", bufs=4) as stgf, \
                 tc.tile_pool(name="dsb", bufs=5) as dsb, \
                 tc.tile_pool(name="ehp", bufs=8) as ehp:
                ones2v = ones2c8.rearrange("p (two s) -> p two s", two=2)

                def prep_slab(tp):
                    """attention prep for batch tp (tokens [tp*1024, ...)).
                    Borrows phase-D psum banks via same-shape tags."""
                    wsl = slice(tp * 1024, (tp + 1) * 1024)
                    for k in range(KU):
                        nc.vector.tensor_scalar_mul(
                            w3x8[:, k, wsl], x2q8[:, k, wsl],
                            aw3s[:, k:k + 1])
                    for jt in range(8 * tp, 8 * tp + 8):
                        ptr = pds.tile([128, 1024], F32, tag="ps",
                                       name=f"ptr_{jt}").bitcast(F8)
                        ptv = ptr[:, 0:2048].rearrange(
                            "p (n two) -> p n two", two=2)
                        for k in range(KU):
                            nc.tensor.transpose(
                                ptv[:, k * 128:(k + 1) * 128, 0:1],
                                x2q8[:, k, jt * 128:(jt + 1) * 128], ident8)
                        if jt % 2 == 0:
                            nc.vector.tensor_copy(xO8[:, jt, :],
                                                  ptv[:, 0:512, 0:1])
                        else:
                            nc.scalar.copy(xO8[:, jt, :],
                                           ptv[:, 0:512, 0:1])
                    for t in (2 * tp, 2 * tp + 1):
                        ps1 = pdr.tile([1, 512], F32, tag="pr",
                                       name=f"ps1_{t}")
                        for kk in range(KU // 2):
                            nc.tensor.matmul(
                                ps1, pair(w1h8, kk, slice(0, 1)),
                                pair(x2q8, kk,
                                     slice(t * 512, (t + 1) * 512)),
                                perf_mode=DR,
                                start=(kk == 0), stop=(kk == 1))
                        nc.scalar.activation(
                            thr[:, t * 512:(t + 1) * 512], ps1, AF.Exp,
                            bias=nab_sb, scale=-1.0 / WSA)
                    s2p = pbc.tile([128, 512], F32, tag="pb",
                                   name=f"s2p_{tp}")
                    for jt in range(8 * tp, 8 * tp + 8):
                        jsl = slice(jt * 128, (jt + 1) * 128)
                        for kk in range(KU // 2):
                            nc.tensor.matmul(
                                s2p[:, jt - 8 * tp:jt - 8 * tp + 1],
                                pair(x2q8, kk, jsl),
                                pair(w2h8, kk, slice(0, 1)),
                                perf_mode=DR,
                                start=(kk == 0), stop=(kk == 1))
                    nc.scalar.mul(s2f[:, 8 * tp:8 * tp + 8],
                                  s2p[:, 0:8], 1.0 / WSA)

                prep_slab(0)
                for b in range(BPC):
                    for h in range(IH):
                        if b == 0 and h == 1:
                            # batch-1 prep overlaps unit (0,1)'s exp chain
                            prep_slab(1)
                        # drip-feed fuse-gate weight loads (DMA idle here)
                        unit = b * IH + h
                        for ci in range(unit * 4, unit * 4 + 4):
                            wv_, wbf_, w8_, k_ = fuse_chunks[ci]
                            wsf = stgf.tile([128, U], F32, tag="wsf",
                                            name=f"wsf_{ci}")
                            nc.sync.dma_start(wsf, wv_[k_])
                            if k_ < KU:
                                if ci % 2 == 0:
                                    nc.vector.tensor_scalar_mul(
                                        wbf_[:, k_, :], wsf, WSF)
                                else:
                                    nc.scalar.mul(wbf_[:, k_, :], wsf, WSF)
                                if k_ < 2:   # fp8 copy for the DR x-half
                                    x8d = ffx8 if ci < 2 * KU else frx8
                                    nc.scalar.copy(x8d[:, k_, :],
                                                   wbf_[:, k_, :])
                            else:
                                if ci % 2 == 0:
                                    nc.vector.tensor_scalar_mul(
                                        w8_[:, k_ - KU, :], wsf, WSH)
                                else:
                                    nc.scalar.mul(w8_[:, k_ - KU, :], wsf,
                                                  WSH)
                        isl = slice(b * L + h * 512, b * L + (h + 1) * 512)
                        # keep-warm burst across the unit boundary
                        wtd = pbc.tile([128, 512], F32, tag="pb",
                                       name=f"wd_{b}_{h}")
                        for i in range(4):
                            nc.tensor.matmul(wtd[:, 0:128], identb, identb,
                                             start=True, stop=True)
                        thbc = dsb.tile([128, 512], BF16, tag="thbc")
                        pb1 = pbc.tile([128, 512], F32, tag="pb",
                                       name=f"pb1_{b}_{h}")
                        nc.tensor.matmul(pb1, ones_row, thr[:, isl],
                                         start=True, stop=True)
                        nc.scalar.copy(thbc, pb1)
                        # ---- stage 1: scores -> exp -> fp8 eh pair tiles
                        ehs = []
                        for p in range(JT // 2):      # j-tile pairs
                            jg = b * JT + 2 * p
                            ps = pds.tile([128, 1024], F32, tag="ps",
                                          name=f"ps_{b}_{h}_{p}")
                            ehb = ehp.tile([128, 1024], F8, tag="ehb",
                                           name=f"ehb_{b}_{h}_{p}")
                            ehbf = dsb.tile([128, 1024], BF16, tag="ehbf")
                            for half in range(2):
                                jsl = slice((jg + half) * 128,
                                            (jg + half + 1) * 128)
                                hsl = slice(half * 512, (half + 1) * 512)
                                for kk in range(KU // 2):
                                    nc.tensor.matmul(
                                        ps[:, hsl], pair(w3x8, kk, jsl),
                                        pair(x2q8, kk, isl), perf_mode=DR,
                                        start=(kk == 0), stop=(kk == 1))
                                nc.scalar.activation(
                                    ehbf[:, hsl], ps[:, hsl], AF.Exp,
                                    bias=s2f[:, jg + half:jg + half + 1],
                                    scale=1.0 / WSA)
                                nc.vector.tensor_tensor(
                                    ehb[:, hsl], ehbf[:, hsl], thbc,
                                    op=OP.max)
                            ehs.append(
                                ehb.rearrange("p (two n) -> p two n", two=2))
                        # ---- stage 2: denominator first, then du-major
                        # numerator accumulation
                        pr = pdr.tile([1, 512], F32, tag="pr")
                        for p in range(JT // 2):
                            nc.tensor.matmul(
                                pr, ones2v[:, :, 0:1], ehs[p], perf_mode=DR,
                                start=(p == 0), stop=(p == JT // 2 - 1))
                        rec = dsb.tile([1, 512], F32, tag="rec")
                        nc.vector.reciprocal_approx_fast(rec, pr)
                        rech = dsb.tile([1, 512], BF16, tag="rech")
                        nc.scalar.mul(rech, rec, ATS)
                        rbc = dsb.tile([128, 512], BF16, tag="rbc")
                        pb2 = pbc.tile([128, 512], F32, tag="pb",
                                       name=f"pb2_{b}_{h}")
                        nc.tensor.matmul(pb2, ones_row, rech,
                                         start=True, stop=True)
                        nc.scalar.copy(rbc, pb2)
                        for du in range(KU):
                            pn = pdn.tile([128, 512], F32, tag="pn",
                                          name=f"pn_{b}_{h}_{du}")
                            for p in range(JT // 2):
                                jg = b * JT + 2 * p
                                nc.tensor.matmul(
                                    pn,
                                    xO8[:, jg:jg + 2,
                                        du * 128:(du + 1) * 128],
                                    ehs[p], perf_mode=DR,
                                    start=(p == 0), stop=(p == JT // 2 - 1))
                            # drain + normalize (*8) in one pass
                            nc.vector.tensor_tensor(
                                attT8[:, du, isl], pn, rbc, op=OP.mult)

            # ============= Phase E: fuse gates + output ====================
            with tc.tile_pool(name="pep", bufs=2, space="PSUM") as pep, \
                 tc.tile_pool(name="peb", bufs=1, space="PSUM") as peb, \
                 tc.tile_pool(name="esb", bufs=3) as esb:
                # broadcast fuse biases (*256) to [128, 512] once
                fbb = esb.tile([128, U], BF16, tag="fbb")
                rbb = esb.tile([128, U], BF16, tag="rbb")
                pfb = peb.tile([128, 512], F32, tag="pfb", name="pfb_f")
                nc.tensor.matmul(pfb, ones_row, ffb_h, start=True, stop=True)
                nc.vector.tensor_copy(fbb, pfb)
                prb = peb.tile([128, 512], F32, tag="pfb", name="pfb_r")
                nc.tensor.matmul(prb, ones_row, frb_h, start=True, stop=True)
                nc.vector.tensor_copy(rbb, prb)
                for mt in range(NT):
                    msl = slice(mt * 128, (mt + 1) * 128)
                    pz = pep.tile([128, 512], F32, tag="pz")
                    pr2 = pep.tile([128, 512], F32, tag="pr2")
                    # x-half: k-tiles 0-1 as one fp8 DR pass, 2-3 bf16
                    nc.tensor.matmul(pz, pair(x0q8, 0, msl), ffx8,
                                     perf_mode=DR, start=True, stop=False)
                    nc.tensor.matmul(pr2, pair(x0q8, 0, msl), frx8,
                                     perf_mode=DR, start=True, stop=False)
                    for k in (2, 3):
                        nc.tensor.matmul(pz, xTh[:, k, msl], ffWx[:, k, :],
                                         start=False, stop=False)
                        nc.tensor.matmul(pr2, xTh[:, k, msl], frWx[:, k, :],
                                         start=False, stop=False)
                    for kk in range(KU // 2):    # att-half, fp8 DR
                        nc.tensor.matmul(pz, pair(attT8, kk, msl),
                                         pair(ffW8, kk), perf_mode=DR,
                                         start=False, stop=(kk == 1))
                        nc.tensor.matmul(pr2, pair(attT8, kk, msl),
                                         pair(frW8, kk), perf_mode=DR,
                                         start=False, stop=(kk == 1))
                    # bias add on vector (frees psum early), sigmoid on
                    # scalar from SBUF
                    pzs = esb.tile([128, U], BF16, tag="pzs")
                    prs = esb.tile([128, U], BF16, tag="prs")
                    nc.vector.tensor_tensor(pzs, pz, fbb, op=OP.add)
                    nc.vector.tensor_tensor(prs, pr2, rbb, op=OP.add)
                    zh = esb.tile([128, U], BF16, tag="zh")
                    rh = esb.tile([128, U], BF16, tag="rh")
                    q = esb.tile([128, U], F32, tag="q")
                    p2 = esb.tile([128, U], F32, tag="p2")
                    ot = esb.tile([128, U], F32, tag="ot")
                    x0t = x0row[:, mt, :]
                    if mt == NT - 1:
                        # shorten the kernel tail: split across engines
                        hU = U // 2
                        nc.scalar.activation(zh, pzs, AF.Sigmoid,
                                             scale=1.0 / WSF)
                        nc.scalar.square(q, zh)
                        nc.scalar.activation(rh, prs, AF.Sigmoid,
                                             scale=1.0 / WSF)
                        nc.vector.tensor_tensor(p2[:, :hU], rh[:, :hU],
                                                x0t[:, :hU], op=OP.mult)
                        nc.gpsimd.tensor_tensor(p2[:, hU:], rh[:, hU:],
                                                x0t[:, hU:], op=OP.mult)
                        nc.vector.tensor_tensor(ot[:, :hU], q[:, :hU],
                                                p2[:, :hU], op=OP.add)
                        nc.gpsimd.tensor_tensor(ot[:, hU:], q[:, hU:],
                                                p2[:, hU:], op=OP.add)
                    else:
                        nc.scalar.activation(zh, pzs, AF.Sigmoid,
                                             scale=1.0 / WSF)
                        nc.scalar.activation(rh, prs, AF.Sigmoid,
                                             scale=1.0 / WSF)
                        nc.scalar.square(q, zh)
                        nc.vector.tensor_tensor(p2, rh, x0t, op=OP.mult)
                        nc.gpsimd.tensor_tensor(ot, q, p2, op=OP.add)
                    nc.sync.dma_start(outv[mt], ot)

    nc.compile()
    return nc


_NC_CACHE = None


def _get_nc():
    global _NC_CACHE
    if _NC_CACHE is None:
        _NC_CACHE = build_nc()
    return _NC_CACHE


def kernel(**inputs) -> np.ndarray:
    from concourse.bass_utils import run_bass_kernel_spmd

    nc = _get_nc()
    full = {k: np.ascontiguousarray(np.asarray(v, dtype=np.float32))
            for k, v in inputs.items()}
    in_maps = []
    for c in range(NCORES):
        m = dict(full)
        m["inputs"] = np.ascontiguousarray(
            full["inputs"][c * BPC:(c + 1) * BPC])
        in_maps.append(m)
    res = run_bass_kernel_spmd(nc, in_maps, core_ids=list(range(NCORES)))
    return np.concatenate([res.results[c]["out"] for c in range(NCORES)],
                          axis=0)
